# revision 70
# baseline (speedup 1.0000x reference)
"""Trainium2 Bass kernel for nn_BlockWithCache (Music-Transformer block w/ rel-pos).

Sharding (8 NeuronCores, uniform SPMD program; per-core differences live in the
input data only):
  - core c: batch element b = c//2, tensor-parallel half = c%2.
  - Attention: TP over heads — each core computes its 8 of 16 heads for the
    full 1024-token sequence (weight column slices supplied by the host).
  - Wproj row-slices produce partial attention outputs; a single pairwise
    bf16 ReduceScatter(add) over [L, D] both completes the sum and splits
    tokens in half.
  - From the residual on: token-split — each core owns 512 tokens through
    LN2 + FFN (full 4*D hidden) and writes a disjoint output half.

Key tricks:
  - bf16 operands everywhere on the matmul paths (weights cast on host),
    fp32 PSUM accumulation; halves weight DMA vs fp32.
  - Music-Transformer skew: QEr rows round-trip through a DRAM buffer (both
    heads interleaved per row so write+read are single DMAs) written with row
    stride 2*srow and read back with row stride 2*srow-1, which realigns
    QEr[q, 1023-q+c] to [q, c]; the 128-wide pad region holds -240 (fp8e4 max
    FINITE -- the format has inf, and an -inf pad would make the 0*(-inf)
    matmul terms NaN) so the causal mask comes back for free (exp -> ~0).
    The buffer is fp8e4 with an 8x prescale folded into Er host-side and
    un-done by a 0.125-diagonal in the Srel-add identity matmul.
  - Softmax without max-subtraction (logits are small), denominator via the
    ACT engine's fused accum_out. The exp tiles stay UNNORMALIZED; the
    1/denominator is applied once at the y = att@V read-out, with the
    per-token reciprocal row replicated across partitions by a stride-0
    partition-broadcast DMA read from DRAM. att@V runs per 512-column half,
    emitted as soon as that half's attT blocks + denominators exist.
  - attT via PE transpose (bf16), copies batched 4 blocks per DVE op; R-phase
    PSUM evacuations split DVE/ACT to balance the two busiest engines.
  - One bf16 ReduceScatter over the full [L, D] proj partials (replaces two
    fp32 collectives); per-token-chunk writes let token 0's residual + LN2
    chain start while later chunks still project.
"""

import os
import sys

os.environ.setdefault("MYCRO_LOCAL_CACHE", "1")
if "/opt/trn_rl_repo" not in sys.path:
    sys.path.insert(0, "/opt/trn_rl_repo")

import numpy as np

B, L, D, H = 4, 1024, 1024, 16
HS = D // H          # 64
P = 128
TC = L // P          # 8 token chunks
DCH = D // P         # 8 feature chunks
NHC = H // 2         # 8 heads per core
FD = 4 * D           # 4096
FC = FD // P         # 32
TMY = L // 2         # 512 tokens owned after RS
T2 = TMY // P        # 4
EPS = 1e-5
SCALE = 1.0 / 8.0    # 1/sqrt(HS)
ERPRE = 8.0          # fp8 skew prescale (undone by 0.125 diag)
NEGF8 = -240.0       # fp8e4 max finite (fmt has inf!); -240*0.125=-30 after
                     # diag -> exp(-30+qk) ~ 1e-11 ~ 0. Must stay finite:
                     # an -inf pad would make the 0*(-inf) matmul terms NaN.
_PROGRAM_CACHE = {}
PHASE_MARKS = []


def _build_program(flags, no_rs=False):
    import concourse.mybir as mybir
    import concourse.tile as tile
    from concourse import bacc
    from concourse.masks import make_identity

    (aff1, aff2, use_bq, use_bk, use_bv, use_bproj, use_bfc, use_bfc2) = flags

    f32 = mybir.dt.float32
    bf16 = mybir.dt.bfloat16
    fp8 = mybir.dt.float8e4
    AF = mybir.ActivationFunctionType
    ALU = mybir.AluOpType
    AX = mybir.AxisListType

    nc = bacc.Bacc("TRN2", target_bir_lowering=False, debug=False, num_devices=8)
    PHASE_MARKS.clear()

    def mark(label):
        PHASE_MARKS.append((label, nc.next_id()))

    x_in = nc.declare_dram_parameter("x", [L, D], bf16, isOutput=False)
    xmy_in = nc.declare_dram_parameter("x_my", [TMY, D], f32, isOutput=False)
    wqkv_in = nc.declare_dram_parameter("wqkv", [D, 3 * NHC * HS], bf16, isOutput=False)
    wproj_in = nc.declare_dram_parameter("wproj", [NHC * HS, D], bf16, isOutput=False)
    ert2_in = nc.declare_dram_parameter("ert2", [P, L], bf16, isOutput=False)
    wfc_in = nc.declare_dram_parameter("wfc", [D, FD], bf16, isOutput=False)
    wfc2_in = nc.declare_dram_parameter("wfc2", [FD, D], bf16, isOutput=False)
    # Always-declared small params (cheap; used only when flags set)
    ln1a_in = nc.declare_dram_parameter("ln1a", [D], f32, isOutput=False)
    ln1b_in = nc.declare_dram_parameter("ln1b", [D], f32, isOutput=False)
    ln2a_in = nc.declare_dram_parameter("ln2a", [D], f32, isOutput=False)
    ln2b_in = nc.declare_dram_parameter("ln2b", [D], f32, isOutput=False)
    bq_in = nc.declare_dram_parameter("bq", [P, 4], f32, isOutput=False)
    bk_in = nc.declare_dram_parameter("bk", [P, 4], f32, isOutput=False)
    bv_in = nc.declare_dram_parameter("bv", [NHC * HS], f32, isOutput=False)
    bproj_in = nc.declare_dram_parameter("bproj", [D], f32, isOutput=False)
    bfc_in = nc.declare_dram_parameter("bfc", [P, FC], f32, isOutput=False)
    bfc2_in = nc.declare_dram_parameter("bfc2", [D], f32, isOutput=False)

    out_dram = nc.declare_dram_parameter("out_my", [TMY, D], f32, isOutput=True)

    def layernorm(tc, nc, pools, xs, hs, nchunks, aff, wbc, bbc, eps_ap):
        """Per-chunk two-pass LN (bf16 in, bf16 out) so chunk t's output is
        ready without waiting on later chunks."""
        small, scratch = pools
        for t in range(nchunks):
            st = small.tile([P, 8], f32, tag="ln_st")
            # st cols: 0 sum, 1 sumsq, 2 mu, 3 mu^2, 4 var, 5 std, 6 rstd, 7 mur
            nc.vector.reduce_sum(st[:, 0:1], xs[t][:], axis=AX.X)
            sq = scratch.tile([P, D], f32, tag="ln_sq")
            nc.scalar.activation(sq[:], xs[t][:], AF.Square, accum_out=st[:, 1:2])
            nc.vector.tensor_scalar_mul(st[:, 2:3], st[:, 0:1], 1.0 / D)
            nc.vector.tensor_tensor(st[:, 3:4], st[:, 2:3], st[:, 2:3], op=ALU.mult)
            nc.vector.tensor_scalar(
                st[:, 4:5], st[:, 1:2], 1.0 / D, st[:, 3:4],
                op0=ALU.mult, op1=ALU.subtract,
            )
            nc.scalar.activation(st[:, 5:6], st[:, 4:5], AF.Sqrt, bias=eps_ap)
            nc.vector.reciprocal(st[:, 6:7], st[:, 5:6])
            nc.vector.tensor_tensor(st[:, 7:8], st[:, 2:3], st[:, 6:7], op=ALU.mult)
            if aff:
                hf = scratch.tile([P, D], f32, tag="ln_hf")
                nc.vector.tensor_scalar(
                    hf[:], xs[t][:], st[:, 6:7], st[:, 7:8],
                    op0=ALU.mult, op1=ALU.subtract,
                )
                nc.vector.tensor_tensor(hf[:], hf[:], wbc[:], op=ALU.mult)
                nc.vector.tensor_tensor(hs[t][:], hf[:], bbc[:], op=ALU.add)
            else:
                nc.vector.tensor_scalar(
                    hs[t][:], xs[t][:], st[:, 6:7], st[:, 7:8],
                    op0=ALU.mult, op1=ALU.subtract,
                )

    with tile.TileContext(nc) as tc:
        import contextlib

        with contextlib.ExitStack() as es:
            cst = es.enter_context(tc.tile_pool(name="cst", bufs=1))
            small = es.enter_context(tc.tile_pool(name="small", bufs=2))
            dram = es.enter_context(tc.tile_pool(name="dram", bufs=1, space="DRAM"))

            h2Tp = es.enter_context(tc.tile_pool(name="h2Tp", bufs=1))

            # x chunks + residual rows first: these DMAs gate the LN1 pipeline
            # and the HWDGE queue is strictly FIFO per issue order.
            xmp = es.enter_context(tc.tile_pool(name="xmp", bufs=1))
            ysb_pool = tc.alloc_tile_pool(name="ysb", bufs=1)
            qkv_pool = tc.alloc_tile_pool(name="qkv", bufs=1)
            xph = tc.alloc_tile_pool(name="xp", bufs=1)
            xs_tiles = [xph.tile([P, D], bf16, name=f"x{t}") for t in range(TC)]
            for t in range(TC):
                nc.sync.dma_start(xs_tiles[t][:], x_in[t * P : (t + 1) * P, :])
            xm_tiles = [xmp.tile([P, D], f32, name=f"xm{t}") for t in range(T2)]
            for t in range(T2):
                nc.sync.dma_start(xm_tiles[t][:], xmy_in[t * P : (t + 1) * P, :])
            ar_all = xmp.tile([P, T2, D], bf16)
            wproj_sb = [xmp.tile([P, D], bf16, name=f"wpj{p}") for p in range(4)]
            for p in range(4):
                nc.sync.dma_start(wproj_sb[p][:], wproj_in[p * P : (p + 1) * P, :])

            eps_t = cst.tile([P, 1], f32)
            nc.vector.memset(eps_t[:], EPS)
            id16 = cst.tile([P, P], bf16)
            make_identity(nc, id16)
            # fp8 identity with 0.125 diagonal (undoes the 8x Er prescale)
            id8 = cst.tile([P, P], fp8)
            nc.gpsimd.memset(id8[:], 0.0)
            nc.gpsimd.affine_select(
                out=id8[:],
                in_=id8[:],
                compare_op=mybir.AluOpType.not_equal,
                fill=0.125,
                base=0,
                pattern=[[-1, P]],
                channel_multiplier=1,
            )
            ert2 = cst.tile([P, L], bf16)
            nc.sync.dma_start(ert2[:], ert2_in[:])

            ln1w_bc = ln1b_bc = ln2w_bc = ln2b_bc = None
            if aff1:
                row = cst.tile([1, D], f32, tag="lnrow1a")
                nc.sync.dma_start(row[:], ln1a_in[None, :])
                ln1w_bc = cst.tile([P, D], f32)
                nc.gpsimd.partition_broadcast(ln1w_bc[:], row[:])
                row2 = cst.tile([1, D], f32, tag="lnrow1b")
                nc.sync.dma_start(row2[:], ln1b_in[None, :])
                ln1b_bc = cst.tile([P, D], f32)
                nc.gpsimd.partition_broadcast(ln1b_bc[:], row2[:])
            if aff2:
                row = cst.tile([1, D], f32, tag="lnrow2a")
                nc.sync.dma_start(row[:], ln2a_in[None, :])
                ln2w_bc = cst.tile([P, D], f32)
                nc.gpsimd.partition_broadcast(ln2w_bc[:], row[:])
                row2 = cst.tile([1, D], f32, tag="lnrow2b")
                nc.sync.dma_start(row2[:], ln2b_in[None, :])
                ln2b_bc = cst.tile([P, D], f32)
                nc.gpsimd.partition_broadcast(ln2b_bc[:], row2[:])
            bq_sb = bk_sb = None
            if use_bq:
                bq_sb = cst.tile([P, 4], f32)
                nc.sync.dma_start(bq_sb[:], bq_in[:])
            if use_bk:
                bk_sb = cst.tile([P, 4], f32)
                nc.sync.dma_start(bk_sb[:], bk_in[:])
            bv_bc = None
            if use_bv:
                row = cst.tile([1, NHC * HS], f32, tag="bvrow")
                nc.sync.dma_start(row[:], bv_in[None, :])
                bv_bc = cst.tile([P, NHC * HS], f32)
                nc.gpsimd.partition_broadcast(bv_bc[:], row[:])
            bproj_bc = None
            if use_bproj:
                row = cst.tile([1, D], f32, tag="bprow")
                nc.sync.dma_start(row[:], bproj_in[None, :])
                bproj_bc = cst.tile([P, D], f32)
                nc.gpsimd.partition_broadcast(bproj_bc[:], row[:])
            bfc_sb = None
            if use_bfc:
                bfc_sb = cst.tile([P, FC], f32)
                nc.sync.dma_start(bfc_sb[:], bfc_in[:])
            bfc2_bc = None
            if use_bfc2:
                row = cst.tile([1, D], f32, tag="b2row")
                nc.sync.dma_start(row[:], bfc2_in[None, :])
                bfc2_bc = cst.tile([P, D], f32)
                nc.gpsimd.partition_broadcast(bfc2_bc[:], row[:])

            # Proj-partial DRAM buffers for the single bf16 ReduceScatter
            cc_in = dram.tile([L, D], bf16, name="cc_in")
            cc_out = dram.tile([TMY, D], bf16, name="cc_out")
            # reciprocal-row round-trip buffers (one per head-pair parity)
            rcd = [dram.tile([2, L], f32, name=f"rcd{n}") for n in range(2)]

            # Skew DRAM buffers: per q-chunk, 2 slots (fp8), both heads
            # interleaved per row so write+read are single DMAs.
            # Layout: elem (q-row r, head i, col c) at r*2*srow + i*srow + c.
            negpad = cst.tile([P, 2, P], fp8)
            nc.vector.memset(negpad[:], NEGF8)
            skewbufs = []
            for qc in range(TC):
                srow = P * (qc + 2)
                wm = P * (qc + 1)
                slots = []
                for s in range(2):
                    d2 = dram.tile([2 * P * srow], fp8, name=f"skew_{qc}_{s}")
                    wv_full = d2[:].rearrange("(r i c) -> r i c", i=2, c=srow)
                    nc.gpsimd.dma_start(wv_full[:, :, wm:], negpad[:])
                    slots.append(d2)
                skewbufs.append(slots)

            # ---------------- persistent activation tiles ----------------
            ysb = [ysb_pool.tile([P, L], bf16, name=f"ysb{p}") for p in range(4)]

            qt_sb = [qkv_pool.tile([P, L], bf16, name=f"qt{p}") for p in range(4)]
            kt_sb = [qkv_pool.tile([P, L], bf16, name=f"kt{p}") for p in range(4)]
            v_sb = [qkv_pool.tile([P, NHC * HS], bf16, name=f"v{t}") for t in range(TC)]

            mark("ln1")
            # ---------------- LN1 + transpose + QKV ----------------
            with tc.tile_pool(name="hT", bufs=1) as hTp:
                hT = [hTp.tile([P, L], bf16, name=f"hT{d}") for d in range(DCH)]
                with tc.tile_pool(name="xh", bufs=1) as xh, tc.tile_pool(
                    name="lnscr", bufs=2
                ) as lnscr:
                    xs = xs_tiles
                    hs = [xh.tile([P, D], bf16, name=f"h{t}") for t in range(TC)]
                    layernorm(
                        tc, nc, (small, lnscr), xs, hs, TC, aff1, ln1w_bc, ln1b_bc,
                        eps_t[:],
                    )
                    with tc.tile_pool(name="htps", bufs=4, space="PSUM") as htps:
                        for t in range(TC):
                            tp = htps.tile([P, 4, P], bf16, tag="htp")
                            tp2 = htps.tile([P, 4, P], bf16, tag="htp2")
                            for d in range(DCH):
                                dst = tp if d < 4 else tp2
                                nc.tensor.transpose(
                                    dst[:, d % 4, :], hs[t][:, d * P : (d + 1) * P],
                                    id16[:],
                                )
                            for d in range(DCH):
                                dst = tp if d < 4 else tp2
                                nc.vector.tensor_copy(
                                    hT[d][:, t * P : (t + 1) * P], dst[:, d % 4, :]
                                )

                # QKV projections (h freed; hT alive)
                with tc.tile_pool(name="wqkv", bufs=1) as wp, tc.tile_pool(
                    name="qkvps", bufs=4, space="PSUM"
                ) as qps:
                    wqkv_sb = [
                        wp.tile([P, 3 * NHC * HS], bf16, name=f"wqkv{d}")
                        for d in range(DCH)
                    ]
                    for d in range(DCH):
                        nc.sync.dma_start(
                            wqkv_sb[d][:], wqkv_in[d * P : (d + 1) * P, :]
                        )
                    # Q^T and K^T: out [128(2 heads), tokens]
                    for p in range(4):
                        for n in range(2):
                            ps = qps.tile([P, 512], f32, tag="qkvp")
                            for d in range(DCH):
                                nc.tensor.matmul(
                                    ps[:],
                                    wqkv_sb[d][:, p * P : (p + 1) * P],
                                    hT[d][:, n * 512 : (n + 1) * 512],
                                    start=(d == 0),
                                    stop=(d == DCH - 1),
                                )
                            nc.scalar.activation(
                                qt_sb[p][:, n * 512 : (n + 1) * 512],
                                ps[:],
                                AF.Copy,
                                scale=SCALE,
                            )
                            if use_bq:
                                nc.vector.tensor_scalar_add(
                                    qt_sb[p][:, n * 512 : (n + 1) * 512],
                                    qt_sb[p][:, n * 512 : (n + 1) * 512],
                                    bq_sb[:, p : p + 1],
                                )
                        for n in range(2):
                            ps = qps.tile([P, 512], f32, tag="qkvp")
                            for d in range(DCH):
                                nc.tensor.matmul(
                                    ps[:],
                                    wqkv_sb[d][:, 512 + p * P : 512 + (p + 1) * P],
                                    hT[d][:, n * 512 : (n + 1) * 512],
                                    start=(d == 0),
                                    stop=(d == DCH - 1),
                                )
                            nc.scalar.activation(
                                kt_sb[p][:, n * 512 : (n + 1) * 512], ps[:], AF.Copy
                            )
                            if use_bk:
                                nc.vector.tensor_scalar_add(
                                    kt_sb[p][:, n * 512 : (n + 1) * 512],
                                    kt_sb[p][:, n * 512 : (n + 1) * 512],
                                    bk_sb[:, p : p + 1],
                                )
                    # V: out [tokens, 512 hs-cols]
                    for t in range(TC):
                        ps = qps.tile([P, 512], f32, tag="qkvp")
                        for d in range(DCH):
                            nc.tensor.matmul(
                                ps[:],
                                hT[d][:, t * P : (t + 1) * P],
                                wqkv_sb[d][:, 1024:1536],
                                start=(d == 0),
                                stop=(d == DCH - 1),
                            )
                        if use_bv:
                            nc.vector.tensor_tensor(
                                ps[:], ps[:], bv_bc[:], op=ALU.add
                            )
                        nc.scalar.activation(v_sb[t][:], ps[:], AF.Copy)

            xph.release()

            mark("attention")
            # ---------------- attention ----------------
            with contextlib.ExitStack() as att_es:
                expp = att_es.enter_context(tc.tile_pool(name="expp", bufs=8))
                srelp = att_es.enter_context(tc.tile_pool(name="srelp", bufs=4))
                rsbp = att_es.enter_context(tc.tile_pool(name="rsbp", bufs=6))
                attTp = att_es.enter_context(tc.tile_pool(name="attTp", bufs=3))
                dnp = att_es.enter_context(tc.tile_pool(name="dnp", bufs=4))
                rcp = att_es.enter_context(tc.tile_pool(name="rcp", bufs=2))
                sps = att_es.enter_context(tc.tile_pool(name="sps", bufs=3, space="PSUM"))
                rps = att_es.enter_context(tc.tile_pool(name="rps", bufs=1, space="PSUM"))
                tps = att_es.enter_context(tc.tile_pool(name="tps", bufs=2, space="PSUM"))
                yps = att_es.enter_context(tc.tile_pool(name="yps", bufs=1, space="PSUM"))

                def emit_rphase(pr):
                    """R = Q Er^T (both heads, concurrent row groups) -> DRAM
                    skew write -> skewed read (Srel, fp8). Both heads share
                    one interleaved buffer so write+read are single DMAs."""
                    srels = []
                    for qc in range(TC):
                        wp_ = P * (qc + 1)
                        m0 = 896 - P * qc
                        srow = P * (qc + 2)
                        nsub = (wp_ + 511) // 512
                        d2 = skewbufs[qc][pr % 2]
                        base = d2[:]
                        APc = type(base)
                        wview = base.rearrange("(r i c) -> r i c", i=2, c=srow)
                        rview = APc(
                            base.tensor,
                            base.offset + 127,
                            [[2 * srow - 1, P], [srow, 2], [1, wp_]],
                        )
                        rsb = rsbp.tile([P, 2, wp_], fp8, tag="rsb")
                        for i in range(2):
                            off = i * 64
                            lhsq = qt_sb[pr][off : off + 64, qc * P : (qc + 1) * P]
                            for s in range(nsub):
                                w = min(512, wp_ - s * 512)
                                rp = rps.tile([P, 512], f32, tag=f"rp{i}")
                                nc.tensor.matmul(
                                    rp[:, :w],
                                    lhsq,
                                    ert2[off : off + 64, m0 + s * 512 : m0 + s * 512 + w],
                                    start=True,
                                    stop=True,
                                )
                                # engine balance: DVE is the busiest engine
                                # in the attention window, so shift a slice
                                # of the PSUM->SBUF evacuations to ACT
                                if i == 1 and qc >= 6:
                                    nc.scalar.activation(
                                        rsb[:, i, s * 512 : s * 512 + w],
                                        rp[:, :w],
                                        AF.Copy,
                                    )
                                else:
                                    nc.vector.tensor_copy(
                                        rsb[:, i, s * 512 : s * 512 + w], rp[:, :w]
                                    )
                        nc.sync.dma_start(wview[:, :, :wp_], rsb[:])
                        srel = srelp.tile([P, 2, wp_], fp8, tag=f"srel{qc}")
                        nc.sync.dma_start(srel[:], rview)
                        srels.append(srel)
                    return srels

                srel_pending = {0: emit_rphase(0)}
                for pr in range(4):
                    h0, h1 = 2 * pr, 2 * pr + 1
                    if pr + 1 < 4:
                        srel_pending[pr + 1] = emit_rphase(pr + 1)
                    srels2 = srel_pending.pop(pr)
                    attT2 = [
                        attTp.tile([P, TC, L], bf16, tag="attT", name=f"attT_{pr}_{i}")
                        for i in range(2)
                    ]
                    dn = dnp.tile([P, 2, TC, 2], f32, tag="dn")
                    dns = dnp.tile([P, 2, TC], f32, tag="dns")
                    rc = dnp.tile([P, 2, TC], f32, tag="rc")
                    for qc in range(TC):
                        wp_ = P * (qc + 1)     # W' = causal width
                        nsub = (wp_ + 511) // 512
                        lhsq2 = [
                            qt_sb[pr][0:64, qc * P : (qc + 1) * P],
                            qt_sb[pr][64:128, qc * P : (qc + 1) * P],
                        ]
                        exp2 = [
                            expp.tile([P, wp_], bf16, tag="exp", name=f"ex_{pr}_{qc}_{i}")
                            for i in range(2)
                        ]
                        for s in range(nsub):
                            w = min(512, wp_ - s * 512)
                            sl = slice(s * 512, s * 512 + w)
                            sp2 = [
                                sps.tile([P, 512], f32, tag="sp", name=f"sp_{qc}_{s}_{i}")
                                for i in range(2)
                            ]
                            # the two heads' QK matmuls use disjoint PE row
                            # groups (K rows 0-63 vs 64-127) -> run concurrent
                            for i in range(2):
                                nc.tensor.matmul(
                                    sp2[i][:, :w],
                                    lhsq2[i],
                                    kt_sb[pr][64 * i : 64 * i + 64, sl],
                                    start=True,
                                    stop=False,
                                )
                            # += Srel (with fp8-min causal pad) via 0.125-diag
                            # identity matmul (undoes the 8x Er prescale)
                            for i in range(2):
                                nc.tensor.matmul(
                                    sp2[i][:, :w],
                                    id8[:],
                                    srels2[qc][:, i, sl],
                                    start=False,
                                    stop=True,
                                )
                            for i in range(2):
                                nc.scalar.activation(
                                    exp2[i][:, sl], sp2[i][:, :w], AF.Exp,
                                    accum_out=dn[:, i, qc, s : s + 1],
                                )
                        for i in range(2):
                            if nsub == 2:
                                nc.vector.tensor_tensor(
                                    dns[:, i, qc : qc + 1],
                                    dn[:, i, qc, 0:1],
                                    dn[:, i, qc, 1:2],
                                    op=ALU.add,
                                )
                            else:
                                nc.vector.tensor_copy(
                                    dns[:, i, qc : qc + 1], dn[:, i, qc, 0:1]
                                )
                            nc.vector.reciprocal(
                                rc[:, i, qc : qc + 1], dns[:, i, qc : qc + 1]
                            )
                        # transpose blocks into attT (batched 4-per-copy)
                        for i in range(2):
                            for c0 in range(0, qc + 1, 4):
                                ncc = min(4, qc + 1 - c0)
                                tp4 = tps.tile([P, 4, P], bf16, tag="tp4")
                                for j in range(ncc):
                                    nc.tensor.transpose(
                                        tp4[:, j, :],
                                        exp2[i][:, (c0 + j) * P : (c0 + j + 1) * P],
                                        id16[:],
                                    )
                                nc.vector.tensor_copy(
                                    attT2[i][:, c0 : c0 + ncc, qc * P : (qc + 1) * P],
                                    tp4[:, 0:ncc, :],
                                )
                        if qc == 3 or qc == 7:
                            # half of att@V + its normalization, emitted as
                            # soon as the needed attT blocks + denominators
                            # exist: n-half 0 after qc 3, n-half 1 after qc 7
                            nh = 0 if qc == 3 else 1
                            n0h, n1h = nh * 512, nh * 512 + 512
                            qlo, qhi = nh * 4, nh * 4 + 4
                            rcdb = rcd[pr % 2]
                            rb = rcdb[:]
                            APr = type(rb)
                            for i in range(2):
                                nc.gpsimd.dma_start(
                                    APr(
                                        rb.tensor,
                                        rb.offset + i * L + n0h,
                                        [[1, P], [P, 4]],
                                    ),
                                    rc[:, i, qlo:qhi],
                                )
                            rcbc = rcp.tile([P, 512], f32, tag=f"rcbc{nh}")
                            nc.sync.dma_start(
                                rcbc[0:64, :],
                                APr(rb.tensor, rb.offset + n0h, [[0, 64], [1, 512]]),
                            )
                            nc.sync.dma_start(
                                rcbc[64:128, :],
                                APr(rb.tensor, rb.offset + L + n0h, [[0, 64], [1, 512]]),
                            )
                            yp = yps.tile([P, 512], f32, tag="yp")
                            ccs = [c for c in range(TC) if c * P < n1h]
                            for cc in ccs:
                                lo = max(cc * P, n0h)
                                w = n1h - lo
                                nc.tensor.matmul(
                                    yp[0:64, lo - n0h : 512],
                                    v_sb[cc][:, h0 * 64 : h0 * 64 + 64],
                                    attT2[0][:, cc, lo:n1h],
                                    start=(cc == 0),
                                    stop=(cc == ccs[-1]),
                                )
                                nc.tensor.matmul(
                                    yp[64:128, lo - n0h : 512],
                                    v_sb[cc][:, h1 * 64 : h1 * 64 + 64],
                                    attT2[1][:, cc, lo:n1h],
                                    start=(cc == 0),
                                    stop=(cc == ccs[-1]),
                                    tile_position=(0, 64),
                                )
                            nc.vector.tensor_tensor(
                                ysb[pr][:, n0h:n1h], yp[:], rcbc[:], op=ALU.mult
                            )

            qkv_pool.release()

            mark("proj_rs")
            # ---------------- proj (partial) + single bf16 ReduceScatter ----
            with tc.tile_pool(
                name="asb", bufs=1
            ) as asbp, tc.tile_pool(name="aps", bufs=4, space="PSUM") as apsp:
                # dummy sqrt whose output feeds LN2's eps: forces the sqrt
                # table load to happen HERE (ACT idle) instead of on the
                # LN2 critical chain
                eps2 = cst.tile([P, 1], f32)
                nc.scalar.activation(eps2[:], eps_t[:], AF.Sqrt)
                nc.scalar.activation(eps2[:], eps2[:], AF.Square)
                asb = asbp.tile([P, TC, D], bf16)
                for t in range(TC):
                    for n in range(2):
                        ap_ = apsp.tile([P, 512], f32, tag="ap")
                        for p in range(4):
                            nc.tensor.matmul(
                                ap_[:],
                                ysb[p][:, t * P : (t + 1) * P],
                                wproj_sb[p][:, n * 512 : (n + 1) * 512],
                                start=(p == 0),
                                stop=(p == 3),
                            )
                        nc.scalar.activation(
                            asb[:, t, n * 512 : (n + 1) * 512], ap_[:], AF.Copy
                        )
                    # one row-block write per token chunk, fired as soon as
                    # that chunk's projection completes
                    nc.gpsimd.dma_start(
                        cc_in[t * P : (t + 1) * P, :], asb[:, t, :]
                    )
                    if no_rs and t < T2:
                        # stand-in for the RS: copy + readback fired per
                        # chunk so token 0's residual/LN2 chain starts while
                        # later chunks still project
                        nc.sync.dma_start(
                            cc_out[t * P : (t + 1) * P, :],
                            cc_in[t * P : (t + 1) * P, :],
                        )
                        nc.sync.dma_start(
                            ar_all[:, t, :], cc_out[t * P : (t + 1) * P, :]
                        )
                if no_rs:
                    pass
                else:
                    nc.gpsimd.collective_compute(
                        "ReduceScatter",
                        mybir.AluOpType.add,
                        replica_groups=[[0, 1], [2, 3], [4, 5], [6, 7]],
                        ins=[cc_in[:]],
                        outs=[cc_out[:]],
                    )
            ysb_pool.release()

            mark("ln2")
            # ---------------- residual + LN2 + h2T ----------------
            x2p = es.enter_context(tc.tile_pool(name="x2p", bufs=1))
            x2 = [x2p.tile([P, D], f32, name=f"x2_{t}") for t in range(T2)]
            h2T = [h2Tp.tile([P, TMY], bf16, name=f"h2T{d}") for d in range(DCH)]
            with tc.tile_pool(name="res", bufs=2) as resp, tc.tile_pool(
                name="lnscr2", bufs=2
            ) as lnscr2:
                h2 = [resp.tile([P, D], bf16, name=f"h2_{t}", bufs=1) for t in range(T2)]
                if not no_rs:
                    nc.sync.dma_start(
                        ar_all[:], cc_out[:].rearrange("(t p) c -> p t c", p=P)
                    )
                for t in range(T2):
                    nc.vector.tensor_tensor(
                        x2[t][:], xm_tiles[t][:], ar_all[:, t, :], op=ALU.add
                    )
                    if use_bproj:
                        nc.vector.tensor_tensor(
                            x2[t][:], x2[t][:], bproj_bc[:], op=ALU.add
                        )
                layernorm(
                    tc, nc, (small, lnscr2), x2, h2, T2, aff2, ln2w_bc, ln2b_bc,
                    eps2[:],
                )
                with tc.tile_pool(name="h2ps", bufs=2, space="PSUM") as h2ps:
                    for t in range(T2):
                        tp = h2ps.tile([P, 4, P], bf16, tag="h2p")
                        tp2 = h2ps.tile([P, 4, P], bf16, tag="h2p2")
                        for d in range(DCH):
                            dst = tp if d < 4 else tp2
                            nc.tensor.transpose(
                                dst[:, d % 4, :], h2[t][:, d * P : (d + 1) * P],
                                id16[:],
                            )
                        for d in range(DCH):
                            dst = tp if d < 4 else tp2
                            nc.vector.tensor_copy(
                                h2T[d][:, t * P : (t + 1) * P], dst[:, d % 4, :]
                            )

            mark("fc1")
            # ---------------- FFN ----------------
            m1p = es.enter_context(tc.tile_pool(name="m1p", bufs=1))
            m1T = [m1p.tile([P, TMY], bf16, name=f"m1T{f}") for f in range(FC)]
            with tc.tile_pool(name="wfcp", bufs=4) as wfcp, tc.tile_pool(
                name="fc1ps", bufs=4, space="PSUM"
            ) as fc1ps:
                for half in range(2):
                    # one [128, 2048] weight tile per d-chunk covers 16
                    # f-chunks (4KB contiguous rows -> efficient DMA)
                    wts = []
                    for d in range(DCH):
                        wt = wfcp.tile([P, 2048], bf16, tag=f"wfc{d % 4}")
                        nc.sync.dma_start(
                            wt[:],
                            wfc_in[d * P : (d + 1) * P,
                                   half * 2048 : (half + 1) * 2048],
                        )
                        wts.append(wt)
                    for fl in range(16):
                        f = half * 16 + fl
                        mp = fc1ps.tile([P, TMY], f32, tag="m1ps")
                        # token-halved rhs so the first matmuls only need
                        # LN2 chunks 0-1 (chunks 2-3 may still be in flight)
                        for th in range(2):
                            tsl = slice(th * 256, th * 256 + 256)
                            for d in range(DCH):
                                nc.tensor.matmul(
                                    mp[:, tsl],
                                    wts[d][:, fl * P : (fl + 1) * P],
                                    h2T[d][:, tsl],
                                    start=(d == 0),
                                    stop=(d == DCH - 1),
                                )
                        if use_bfc:
                            nc.scalar.activation(
                                m1T[f][:], mp[:], AF.Gelu, bias=bfc_sb[:, f : f + 1]
                            )
                        else:
                            nc.scalar.activation(m1T[f][:], mp[:], AF.Gelu)

            mark("fc2")
            # fc2 t-major with fully-resident weights (loaded during fc1) so
            # each token chunk's output lands early and its DMA overlaps the
            # remaining matmuls.
            with tc.tile_pool(name="wfc2p", bufs=1) as wfc2p, tc.tile_pool(
                name="outp", bufs=1
            ) as outp, tc.tile_pool(name="fc2ps", bufs=1, space="PSUM") as fc2ps:
                w2 = [wfc2p.tile([P, D], bf16, name=f"w2_{f}") for f in range(FC)]
                for f in range(FC):
                    nc.sync.dma_start(w2[f][:], wfc2_in[f * P : (f + 1) * P, :])
                out_sb = [outp.tile([P, D], f32, name=f"o{t}") for t in range(T2)]
                pss = [
                    [fc2ps.tile([P, 512], f32, name=f"fc2_{t}_{n}") for n in range(2)]
                    for t in range(T2)
                ]
                for t in range(T2):
                    for f in range(FC):
                        for n in range(2):
                            nc.tensor.matmul(
                                pss[t][n][:],
                                m1T[f][:, t * P : (t + 1) * P],
                                w2[f][:, n * 512 : (n + 1) * 512],
                                start=(f == 0),
                                stop=(f == FC - 1),
                            )
                    for n in range(2):
                        nc.vector.tensor_tensor(
                            out_sb[t][:, n * 512 : (n + 1) * 512],
                            pss[t][n][:],
                            x2[t][:, n * 512 : (n + 1) * 512],
                            op=ALU.add,
                        )
                    if use_bfc2:
                        nc.vector.tensor_tensor(
                            out_sb[t][:], out_sb[t][:], bfc2_bc[:], op=ALU.add
                        )
                    for n in range(2):
                        nc.sync.dma_start(
                            out_dram[t * P : (t + 1) * P, n * 512 : (n + 1) * 512],
                            out_sb[t][:, n * 512 : (n + 1) * 512],
                        )

    mark("end")
    nc.compile()
    return nc


def _get_program(flags):
    if flags not in _PROGRAM_CACHE:
        _PROGRAM_CACHE[flags] = _build_program(flags)
    return _PROGRAM_CACHE[flags]


def kernel(
    x,
    ln1_w,
    ln1_b,
    Wqkv,
    bqkv,
    Wproj,
    bproj,
    Er,
    ln2_w,
    ln2_b,
    Wfc,
    bfc,
    Wfc2,
    bfc2,
):
    import ml_dtypes

    from concourse.bass_utils import run_bass_kernel_spmd

    bf = ml_dtypes.bfloat16
    x = np.asarray(x, np.float32)
    f = np.float32
    ntriv = lambda a, v: not np.all(np.asarray(a) == v)
    flags = (
        ntriv(ln1_w, 1) or ntriv(ln1_b, 0),
        ntriv(ln2_w, 1) or ntriv(ln2_b, 0),
        ntriv(bqkv[:D], 0),
        ntriv(bqkv[D : 2 * D], 0),
        ntriv(bqkv[2 * D :], 0),
        ntriv(bproj, 0),
        ntriv(bfc, 0),
        ntriv(bfc2, 0),
    )
    nc = _get_program(flags)

    ErT = np.asarray(Er, f).T * ERPRE           # [HS, L], 8x prescale
    ert2 = np.ascontiguousarray(
        np.concatenate([ErT, ErT], axis=0)
    ).astype(bf)
    c = np.ascontiguousarray
    Wqkv = np.asarray(Wqkv, f)
    Wfc_b = np.asarray(Wfc, f).astype(bf)
    Wfc2_b = np.asarray(Wfc2, f).astype(bf)
    in_maps = []
    for core in range(8):
        b, half = divmod(core, 2)
        hs0, hs1 = half * 512, (half + 1) * 512
        bq = np.asarray(bqkv[:D][hs0:hs1], f) * SCALE
        bk = np.asarray(bqkv[D : 2 * D][hs0:hs1], f)
        wqkv_half = np.concatenate(
            [
                Wqkv[:, 0:D][:, hs0:hs1],
                Wqkv[:, D : 2 * D][:, hs0:hs1],
                Wqkv[:, 2 * D :][:, hs0:hs1],
            ],
            axis=1,
        ).astype(bf)
        in_maps.append(
            {
                "x": c(x[b]).astype(bf),
                "x_my": c(x[b, hs0:hs1], f),
                "wqkv": c(wqkv_half),
                "wproj": c(np.asarray(Wproj, f)[hs0:hs1, :]).astype(bf),
                "ert2": ert2,
                "wfc": Wfc_b,
                "wfc2": Wfc2_b,
                "ln1a": c(np.asarray(ln1_w), f),
                "ln1b": c(np.asarray(ln1_b), f),
                "ln2a": c(np.asarray(ln2_w), f),
                "ln2b": c(np.asarray(ln2_b), f),
                "bq": c(bq.reshape(4, P).T, f),
                "bk": c(bk.reshape(4, P).T, f),
                "bv": c(np.asarray(bqkv[2 * D :][hs0:hs1]), f),
                "bproj": c(np.asarray(bproj), f),
                "bfc": c(np.asarray(bfc).reshape(FC, P).T, f),
                "bfc2": c(np.asarray(bfc2), f),
            }
        )

    trace = bool(int(os.environ.get("KERNEL_TRACE", "0")))
    res = run_bass_kernel_spmd(nc, in_maps, list(range(8)), trace=trace)
    global LAST_EXEC_NS, LAST_RESULT
    LAST_EXEC_NS = res.exec_time_ns
    LAST_RESULT = res
    out = np.empty((B, L, D), np.float32)
    for core in range(8):
        b, half = divmod(core, 2)
        out[b, half * 512 : (half + 1) * 512] = res.results[core]["out_my"]
    return out


LAST_EXEC_NS = None
LAST_RESULT = None


# revision 85
# speedup vs baseline: 1.0185x; 1.0185x over previous
"""Trainium2 Bass kernel for nn_BlockWithCache (Music-Transformer block w/ rel-pos).

Sharding (8 NeuronCores, uniform SPMD program; per-core differences live in the
input data only):
  - core c: batch element b = c//2, tensor-parallel half = c%2.
  - Attention: TP over heads — each core computes its 8 of 16 heads for the
    full 1024-token sequence (weight column slices supplied by the host).
  - Wproj row-slices produce partial attention outputs; a single pairwise
    bf16 ReduceScatter(add) over [L, D] both completes the sum and splits
    tokens in half.
  - From the residual on: token-split — each core owns 512 tokens through
    LN2 + FFN (full 4*D hidden) and writes a disjoint output half.

Key tricks:
  - bf16 operands everywhere on the matmul paths (weights cast on host),
    fp32 PSUM accumulation; halves weight DMA vs fp32.
  - Music-Transformer skew: QEr rows round-trip through a DRAM buffer (both
    heads interleaved per row so write+read are single DMAs) written with row
    stride 2*srow and read back with row stride 2*srow-1, which realigns
    QEr[q, 1023-q+c] to [q, c]; the 128-wide pad region holds -240 (fp8e4 max
    FINITE -- the format has inf, and an -inf pad would make the 0*(-inf)
    matmul terms NaN) so the causal mask comes back for free (exp -> ~0).
    The buffer is fp8e4 with an 8x prescale folded into Er host-side and
    un-done by a 0.125-diagonal in the Srel-add identity matmul.
  - Softmax without max-subtraction (logits are small), denominator via the
    ACT engine's fused accum_out. The exp tiles stay UNNORMALIZED; the
    1/denominator is applied once at the y = att@V read-out, with the
    per-token reciprocal row replicated across partitions by a stride-0
    partition-broadcast DMA read from DRAM. att@V runs per 512-column half,
    emitted as soon as that half's attT blocks + denominators exist.
  - attT via PE transpose (bf16), copies batched 4 blocks per DVE op; R-phase
    PSUM evacuations split DVE/ACT to balance the two busiest engines.
  - One bf16 ReduceScatter over the full [L, D] proj partials (replaces two
    fp32 collectives); per-token-chunk writes let token 0's residual + LN2
    chain start while later chunks still project.
"""

import os
import sys

os.environ.setdefault("MYCRO_LOCAL_CACHE", "1")
if "/opt/trn_rl_repo" not in sys.path:
    sys.path.insert(0, "/opt/trn_rl_repo")

import numpy as np

B, L, D, H = 4, 1024, 1024, 16
HS = D // H          # 64
P = 128
TC = L // P          # 8 token chunks
DCH = D // P         # 8 feature chunks
NHC = H // 2         # 8 heads per core
FD = 4 * D           # 4096
FC = FD // P         # 32
TMY = L // 2         # 512 tokens owned after RS
T2 = TMY // P        # 4
EPS = 1e-5
SCALE = 1.0 / 8.0    # 1/sqrt(HS)
ERPRE = 8.0          # fp8 skew prescale (undone by 0.125 diag)
NEGF8 = -240.0       # fp8e4 max finite (fmt has inf!); -240*0.125=-30 after
                     # diag -> exp(-30+qk) ~ 1e-11 ~ 0. Must stay finite:
                     # an -inf pad would make the 0*(-inf) matmul terms NaN.
_PROGRAM_CACHE = {}
PHASE_MARKS = []


def _build_program(flags, no_rs=False):
    import concourse.mybir as mybir
    import concourse.tile as tile
    from concourse import bacc
    from concourse.masks import make_identity

    (aff1, aff2, use_bq, use_bk, use_bv, use_bproj, use_bfc, use_bfc2) = flags

    f32 = mybir.dt.float32
    bf16 = mybir.dt.bfloat16
    fp8 = mybir.dt.float8e4
    AF = mybir.ActivationFunctionType
    ALU = mybir.AluOpType
    AX = mybir.AxisListType

    nc = bacc.Bacc("TRN2", target_bir_lowering=False, debug=False, num_devices=8)
    PHASE_MARKS.clear()

    def mark(label):
        PHASE_MARKS.append((label, nc.next_id()))

    x_in = nc.declare_dram_parameter("x", [L, D], bf16, isOutput=False)
    xmy_in = nc.declare_dram_parameter("x_my", [TMY, D], f32, isOutput=False)
    wqkv_in = nc.declare_dram_parameter("wqkv", [D, 3 * NHC * HS], bf16, isOutput=False)
    wproj_in = nc.declare_dram_parameter("wproj", [NHC * HS, D], bf16, isOutput=False)
    ert2_in = nc.declare_dram_parameter("ert2", [P, L], bf16, isOutput=False)
    wfc_in = nc.declare_dram_parameter("wfc", [D, FD], bf16, isOutput=False)
    wfc2_in = nc.declare_dram_parameter("wfc2", [FD, D], bf16, isOutput=False)
    # Always-declared small params (cheap; used only when flags set)
    ln1a_in = nc.declare_dram_parameter("ln1a", [D], f32, isOutput=False)
    ln1b_in = nc.declare_dram_parameter("ln1b", [D], f32, isOutput=False)
    ln2a_in = nc.declare_dram_parameter("ln2a", [D], f32, isOutput=False)
    ln2b_in = nc.declare_dram_parameter("ln2b", [D], f32, isOutput=False)
    bq_in = nc.declare_dram_parameter("bq", [P, 4], f32, isOutput=False)
    bk_in = nc.declare_dram_parameter("bk", [P, 4], f32, isOutput=False)
    bv_in = nc.declare_dram_parameter("bv", [NHC * HS], f32, isOutput=False)
    bproj_in = nc.declare_dram_parameter("bproj", [D], f32, isOutput=False)
    bfc_in = nc.declare_dram_parameter("bfc", [P, FC], f32, isOutput=False)
    bfc2_in = nc.declare_dram_parameter("bfc2", [D], f32, isOutput=False)

    out_dram = nc.declare_dram_parameter("out_my", [TMY, D], f32, isOutput=True)

    def layernorm(tc, nc, pools, xs, hs, nchunks, aff, wbc, bbc, eps_ap):
        """Per-chunk two-pass LN (bf16 in, bf16 out) so chunk t's output is
        ready without waiting on later chunks."""
        small, scratch = pools
        for t in range(nchunks):
            st = small.tile([P, 8], f32, tag="ln_st")
            # st cols: 0 sum, 1 sumsq, 2 mu, 3 mu^2, 4 var, 5 std, 6 rstd, 7 mur
            nc.vector.reduce_sum(st[:, 0:1], xs[t][:], axis=AX.X)
            sq = scratch.tile([P, D], f32, tag="ln_sq")
            nc.scalar.activation(sq[:], xs[t][:], AF.Square, accum_out=st[:, 1:2])
            nc.vector.tensor_scalar_mul(st[:, 2:3], st[:, 0:1], 1.0 / D)
            nc.vector.tensor_tensor(st[:, 3:4], st[:, 2:3], st[:, 2:3], op=ALU.mult)
            nc.vector.tensor_scalar(
                st[:, 4:5], st[:, 1:2], 1.0 / D, st[:, 3:4],
                op0=ALU.mult, op1=ALU.subtract,
            )
            nc.scalar.activation(st[:, 5:6], st[:, 4:5], AF.Sqrt, bias=eps_ap)
            nc.vector.reciprocal(st[:, 6:7], st[:, 5:6])
            nc.vector.tensor_tensor(st[:, 7:8], st[:, 2:3], st[:, 6:7], op=ALU.mult)
            if aff:
                hf = scratch.tile([P, D], f32, tag="ln_hf")
                nc.vector.tensor_scalar(
                    hf[:], xs[t][:], st[:, 6:7], st[:, 7:8],
                    op0=ALU.mult, op1=ALU.subtract,
                )
                nc.vector.tensor_tensor(hf[:], hf[:], wbc[:], op=ALU.mult)
                nc.vector.tensor_tensor(hs[t][:], hf[:], bbc[:], op=ALU.add)
            else:
                nc.vector.tensor_scalar(
                    hs[t][:], xs[t][:], st[:, 6:7], st[:, 7:8],
                    op0=ALU.mult, op1=ALU.subtract,
                )

    with tile.TileContext(nc) as tc:
        import contextlib

        with contextlib.ExitStack() as es:
            cst = es.enter_context(tc.tile_pool(name="cst", bufs=1))
            small = es.enter_context(tc.tile_pool(name="small", bufs=2))
            dram = es.enter_context(tc.tile_pool(name="dram", bufs=1, space="DRAM"))

            h2Tp = es.enter_context(tc.tile_pool(name="h2Tp", bufs=1))

            # x chunks + residual rows first: these DMAs gate the LN1 pipeline
            # and the HWDGE queue is strictly FIFO per issue order.
            xmp = es.enter_context(tc.tile_pool(name="xmp", bufs=1))
            ysb_pool = tc.alloc_tile_pool(name="ysb", bufs=1)
            qkv_pool = tc.alloc_tile_pool(name="qkv", bufs=1)
            xph = tc.alloc_tile_pool(name="xp", bufs=1)
            xs_tiles = [xph.tile([P, D], bf16, name=f"x{t}") for t in range(TC)]
            for t in range(TC):
                nc.sync.dma_start(xs_tiles[t][:], x_in[t * P : (t + 1) * P, :])
            xm_tiles = [xmp.tile([P, D], f32, name=f"xm{t}") for t in range(T2)]
            for t in range(T2):
                nc.sync.dma_start(xm_tiles[t][:], xmy_in[t * P : (t + 1) * P, :])
            ar_all = xmp.tile([P, T2, D], bf16)
            wproj_sb = [xmp.tile([P, D], bf16, name=f"wpj{p}") for p in range(4)]
            for p in range(4):
                nc.sync.dma_start(wproj_sb[p][:], wproj_in[p * P : (p + 1) * P, :])

            eps_t = cst.tile([P, 1], f32)
            nc.vector.memset(eps_t[:], EPS)
            id16 = cst.tile([P, P], bf16)
            make_identity(nc, id16)
            # fp8 identity with 0.125 diagonal (undoes the 8x Er prescale)
            id8 = cst.tile([P, P], fp8)
            nc.gpsimd.memset(id8[:], 0.0)
            nc.gpsimd.affine_select(
                out=id8[:],
                in_=id8[:],
                compare_op=mybir.AluOpType.not_equal,
                fill=0.125,
                base=0,
                pattern=[[-1, P]],
                channel_multiplier=1,
            )
            ert2 = cst.tile([P, L], bf16)
            nc.sync.dma_start(ert2[:], ert2_in[:])

            ln1w_bc = ln1b_bc = ln2w_bc = ln2b_bc = None
            if aff1:
                row = cst.tile([1, D], f32, tag="lnrow1a")
                nc.sync.dma_start(row[:], ln1a_in[None, :])
                ln1w_bc = cst.tile([P, D], f32)
                nc.gpsimd.partition_broadcast(ln1w_bc[:], row[:])
                row2 = cst.tile([1, D], f32, tag="lnrow1b")
                nc.sync.dma_start(row2[:], ln1b_in[None, :])
                ln1b_bc = cst.tile([P, D], f32)
                nc.gpsimd.partition_broadcast(ln1b_bc[:], row2[:])
            if aff2:
                row = cst.tile([1, D], f32, tag="lnrow2a")
                nc.sync.dma_start(row[:], ln2a_in[None, :])
                ln2w_bc = cst.tile([P, D], f32)
                nc.gpsimd.partition_broadcast(ln2w_bc[:], row[:])
                row2 = cst.tile([1, D], f32, tag="lnrow2b")
                nc.sync.dma_start(row2[:], ln2b_in[None, :])
                ln2b_bc = cst.tile([P, D], f32)
                nc.gpsimd.partition_broadcast(ln2b_bc[:], row2[:])
            bq_sb = bk_sb = None
            if use_bq:
                bq_sb = cst.tile([P, 4], f32)
                nc.sync.dma_start(bq_sb[:], bq_in[:])
            if use_bk:
                bk_sb = cst.tile([P, 4], f32)
                nc.sync.dma_start(bk_sb[:], bk_in[:])
            bv_bc = None
            if use_bv:
                row = cst.tile([1, NHC * HS], f32, tag="bvrow")
                nc.sync.dma_start(row[:], bv_in[None, :])
                bv_bc = cst.tile([P, NHC * HS], f32)
                nc.gpsimd.partition_broadcast(bv_bc[:], row[:])
            bproj_bc = None
            if use_bproj:
                row = cst.tile([1, D], f32, tag="bprow")
                nc.sync.dma_start(row[:], bproj_in[None, :])
                bproj_bc = cst.tile([P, D], f32)
                nc.gpsimd.partition_broadcast(bproj_bc[:], row[:])
            bfc_sb = None
            if use_bfc:
                bfc_sb = cst.tile([P, FC], f32)
                nc.sync.dma_start(bfc_sb[:], bfc_in[:])
            bfc2_bc = None
            if use_bfc2:
                row = cst.tile([1, D], f32, tag="b2row")
                nc.sync.dma_start(row[:], bfc2_in[None, :])
                bfc2_bc = cst.tile([P, D], f32)
                nc.gpsimd.partition_broadcast(bfc2_bc[:], row[:])

            # Proj-partial DRAM buffers for the single bf16 ReduceScatter
            cc_in = dram.tile([L, D], bf16, name="cc_in")
            cc_out = dram.tile([TMY, D], bf16, name="cc_out")
            # reciprocal-row round-trip buffers (one per head-pair parity)
            rcd = [dram.tile([2, L], f32, name=f"rcd{n}") for n in range(2)]

            # Skew DRAM buffers: per q-chunk, 2 slots (fp8), both heads
            # interleaved per row so write+read are single DMAs.
            # Layout: elem (q-row r, head i, col c) at r*2*srow + i*srow + c.
            negpad = cst.tile([P, 2, P], fp8)
            nc.vector.memset(negpad[:], NEGF8)
            skewbufs = []
            for qc in range(TC):
                srow = P * (qc + 2)
                wm = P * (qc + 1)
                slots = []
                for s in range(2):
                    d2 = dram.tile([2 * P * srow], fp8, name=f"skew_{qc}_{s}")
                    wv_full = d2[:].rearrange("(r i c) -> r i c", i=2, c=srow)
                    nc.gpsimd.dma_start(wv_full[:, :, wm:], negpad[:])
                    slots.append(d2)
                skewbufs.append(slots)

            # ---------------- persistent activation tiles ----------------
            ysb = [ysb_pool.tile([P, L], bf16, name=f"ysb{p}") for p in range(4)]

            qt_sb = [qkv_pool.tile([P, L], bf16, name=f"qt{p}") for p in range(4)]
            kt_sb = [qkv_pool.tile([P, L], bf16, name=f"kt{p}") for p in range(4)]
            v_sb = [qkv_pool.tile([P, NHC * HS], bf16, name=f"v{t}") for t in range(TC)]

            mark("ln1")
            # ---------------- LN1 + transpose + QKV ----------------
            with tc.tile_pool(name="hT", bufs=1) as hTp:
                hT = [hTp.tile([P, L], bf16, name=f"hT{d}") for d in range(DCH)]
                with tc.tile_pool(name="xh", bufs=1) as xh, tc.tile_pool(
                    name="lnscr", bufs=2
                ) as lnscr:
                    xs = xs_tiles
                    hs = [xh.tile([P, D], bf16, name=f"h{t}") for t in range(TC)]
                    layernorm(
                        tc, nc, (small, lnscr), xs, hs, TC, aff1, ln1w_bc, ln1b_bc,
                        eps_t[:],
                    )
                    with tc.tile_pool(name="htps", bufs=4, space="PSUM") as htps:
                        for t in range(TC):
                            tp = htps.tile([P, 4, P], bf16, tag="htp")
                            tp2 = htps.tile([P, 4, P], bf16, tag="htp2")
                            for d in range(DCH):
                                dst = tp if d < 4 else tp2
                                nc.tensor.transpose(
                                    dst[:, d % 4, :], hs[t][:, d * P : (d + 1) * P],
                                    id16[:],
                                )
                            for d in range(DCH):
                                dst = tp if d < 4 else tp2
                                nc.vector.tensor_copy(
                                    hT[d][:, t * P : (t + 1) * P], dst[:, d % 4, :]
                                )

                # QKV projections (h freed; hT alive)
                with tc.tile_pool(name="wqkv", bufs=1) as wp, tc.tile_pool(
                    name="qkvps", bufs=4, space="PSUM"
                ) as qps:
                    wqkv_sb = [
                        wp.tile([P, 3 * NHC * HS], bf16, name=f"wqkv{d}")
                        for d in range(DCH)
                    ]
                    for d in range(DCH):
                        nc.sync.dma_start(
                            wqkv_sb[d][:], wqkv_in[d * P : (d + 1) * P, :]
                        )
                    # Q^T and K^T: out [128(2 heads), tokens]
                    for p in range(4):
                        for n in range(2):
                            ps = qps.tile([P, 512], f32, tag="qkvp")
                            for d in range(DCH):
                                nc.tensor.matmul(
                                    ps[:],
                                    wqkv_sb[d][:, p * P : (p + 1) * P],
                                    hT[d][:, n * 512 : (n + 1) * 512],
                                    start=(d == 0),
                                    stop=(d == DCH - 1),
                                )
                            nc.scalar.activation(
                                qt_sb[p][:, n * 512 : (n + 1) * 512],
                                ps[:],
                                AF.Copy,
                                scale=SCALE,
                            )
                            if use_bq:
                                nc.vector.tensor_scalar_add(
                                    qt_sb[p][:, n * 512 : (n + 1) * 512],
                                    qt_sb[p][:, n * 512 : (n + 1) * 512],
                                    bq_sb[:, p : p + 1],
                                )
                        for n in range(2):
                            ps = qps.tile([P, 512], f32, tag="qkvp")
                            for d in range(DCH):
                                nc.tensor.matmul(
                                    ps[:],
                                    wqkv_sb[d][:, 512 + p * P : 512 + (p + 1) * P],
                                    hT[d][:, n * 512 : (n + 1) * 512],
                                    start=(d == 0),
                                    stop=(d == DCH - 1),
                                )
                            nc.scalar.activation(
                                kt_sb[p][:, n * 512 : (n + 1) * 512], ps[:], AF.Copy
                            )
                            if use_bk:
                                nc.vector.tensor_scalar_add(
                                    kt_sb[p][:, n * 512 : (n + 1) * 512],
                                    kt_sb[p][:, n * 512 : (n + 1) * 512],
                                    bk_sb[:, p : p + 1],
                                )
                    # V: out [tokens, 512 hs-cols]
                    for t in range(TC):
                        ps = qps.tile([P, 512], f32, tag="qkvp")
                        for d in range(DCH):
                            nc.tensor.matmul(
                                ps[:],
                                hT[d][:, t * P : (t + 1) * P],
                                wqkv_sb[d][:, 1024:1536],
                                start=(d == 0),
                                stop=(d == DCH - 1),
                            )
                        if use_bv:
                            nc.vector.tensor_tensor(
                                ps[:], ps[:], bv_bc[:], op=ALU.add
                            )
                        nc.scalar.activation(v_sb[t][:], ps[:], AF.Copy)

            xph.release()

            mark("attention")
            # ---------------- attention ----------------
            with contextlib.ExitStack() as att_es:
                expp = att_es.enter_context(tc.tile_pool(name="expp", bufs=8))
                srelp = att_es.enter_context(tc.tile_pool(name="srelp", bufs=4))
                rsbp = att_es.enter_context(tc.tile_pool(name="rsbp", bufs=6))
                attTp = att_es.enter_context(tc.tile_pool(name="attTp", bufs=3))
                dnp = att_es.enter_context(tc.tile_pool(name="dnp", bufs=4))
                rcp = att_es.enter_context(tc.tile_pool(name="rcp", bufs=2))
                sps = att_es.enter_context(tc.tile_pool(name="sps", bufs=3, space="PSUM"))
                rps = att_es.enter_context(tc.tile_pool(name="rps", bufs=1, space="PSUM"))
                tps = att_es.enter_context(tc.tile_pool(name="tps", bufs=2, space="PSUM"))
                yps = att_es.enter_context(tc.tile_pool(name="yps", bufs=1, space="PSUM"))

                def emit_rphase(pr):
                    """R = Q Er^T (both heads, concurrent row groups) -> DRAM
                    skew write -> skewed read (Srel, fp8). Both heads share
                    one interleaved buffer so write+read are single DMAs."""
                    srels = []
                    for qc in range(TC):
                        wp_ = P * (qc + 1)
                        m0 = 896 - P * qc
                        srow = P * (qc + 2)
                        nsub = (wp_ + 511) // 512
                        d2 = skewbufs[qc][pr % 2]
                        base = d2[:]
                        APc = type(base)
                        wview = base.rearrange("(r i c) -> r i c", i=2, c=srow)
                        rview = APc(
                            base.tensor,
                            base.offset + 127,
                            [[2 * srow - 1, P], [srow, 2], [1, wp_]],
                        )
                        rsb = rsbp.tile([P, 2, wp_], fp8, tag="rsb")
                        for i in range(2):
                            off = i * 64
                            lhsq = qt_sb[pr][off : off + 64, qc * P : (qc + 1) * P]
                            for s in range(nsub):
                                w = min(512, wp_ - s * 512)
                                rp = rps.tile([P, 512], f32, tag=f"rp{i}")
                                nc.tensor.matmul(
                                    rp[:, :w],
                                    lhsq,
                                    ert2[off : off + 64, m0 + s * 512 : m0 + s * 512 + w],
                                    start=True,
                                    stop=True,
                                )
                                # engine balance: DVE is the busiest engine
                                # in the attention window, so shift a slice
                                # of the PSUM->SBUF evacuations to ACT
                                if i == 1 and qc >= 6:
                                    nc.scalar.activation(
                                        rsb[:, i, s * 512 : s * 512 + w],
                                        rp[:, :w],
                                        AF.Copy,
                                    )
                                else:
                                    nc.vector.tensor_copy(
                                        rsb[:, i, s * 512 : s * 512 + w], rp[:, :w]
                                    )
                        nc.sync.dma_start(wview[:, :, :wp_], rsb[:])
                        srel = srelp.tile([P, 2, wp_], fp8, tag=f"srel{qc}")
                        nc.sync.dma_start(srel[:], rview)
                        srels.append(srel)
                    return srels

                srel_pending = {0: emit_rphase(0)}
                for pr in range(4):
                    h0, h1 = 2 * pr, 2 * pr + 1
                    if pr + 1 < 4:
                        srel_pending[pr + 1] = emit_rphase(pr + 1)
                    srels2 = srel_pending.pop(pr)
                    attT2 = [
                        attTp.tile([P, TC, L], bf16, tag="attT", name=f"attT_{pr}_{i}")
                        for i in range(2)
                    ]
                    dn = dnp.tile([P, 2, TC, 2], f32, tag="dn")
                    dns = dnp.tile([P, 2, TC], f32, tag="dns")
                    rc = dnp.tile([P, 2, TC], f32, tag="rc")
                    for qc in range(TC):
                        wp_ = P * (qc + 1)     # W' = causal width
                        nsub = (wp_ + 511) // 512
                        lhsq2 = [
                            qt_sb[pr][0:64, qc * P : (qc + 1) * P],
                            qt_sb[pr][64:128, qc * P : (qc + 1) * P],
                        ]
                        exp2 = [
                            expp.tile([P, wp_], bf16, tag="exp", name=f"ex_{pr}_{qc}_{i}")
                            for i in range(2)
                        ]
                        for s in range(nsub):
                            w = min(512, wp_ - s * 512)
                            sl = slice(s * 512, s * 512 + w)
                            sp2 = [
                                sps.tile([P, 512], f32, tag="sp", name=f"sp_{qc}_{s}_{i}")
                                for i in range(2)
                            ]
                            # the two heads' QK matmuls use disjoint PE row
                            # groups (K rows 0-63 vs 64-127) -> run concurrent
                            for i in range(2):
                                nc.tensor.matmul(
                                    sp2[i][:, :w],
                                    lhsq2[i],
                                    kt_sb[pr][64 * i : 64 * i + 64, sl],
                                    start=True,
                                    stop=False,
                                )
                            # += Srel (with fp8-min causal pad) via 0.125-diag
                            # identity matmul (undoes the 8x Er prescale)
                            for i in range(2):
                                nc.tensor.matmul(
                                    sp2[i][:, :w],
                                    id8[:],
                                    srels2[qc][:, i, sl],
                                    start=False,
                                    stop=True,
                                )
                            for i in range(2):
                                nc.scalar.activation(
                                    exp2[i][:, sl], sp2[i][:, :w], AF.Exp,
                                    accum_out=dn[:, i, qc, s : s + 1],
                                )
                        for i in range(2):
                            if nsub == 2:
                                nc.vector.tensor_tensor(
                                    dns[:, i, qc : qc + 1],
                                    dn[:, i, qc, 0:1],
                                    dn[:, i, qc, 1:2],
                                    op=ALU.add,
                                )
                            else:
                                nc.vector.tensor_copy(
                                    dns[:, i, qc : qc + 1], dn[:, i, qc, 0:1]
                                )
                            nc.vector.reciprocal(
                                rc[:, i, qc : qc + 1], dns[:, i, qc : qc + 1]
                            )
                        # transpose blocks into attT (batched 4-per-copy)
                        for i in range(2):
                            for c0 in range(0, qc + 1, 4):
                                ncc = min(4, qc + 1 - c0)
                                tp4 = tps.tile([P, 4, P], bf16, tag="tp4")
                                for j in range(ncc):
                                    nc.tensor.transpose(
                                        tp4[:, j, :],
                                        exp2[i][:, (c0 + j) * P : (c0 + j + 1) * P],
                                        id16[:],
                                    )
                                nc.vector.tensor_copy(
                                    attT2[i][:, c0 : c0 + ncc, qc * P : (qc + 1) * P],
                                    tp4[:, 0:ncc, :],
                                )
                        if qc == 3 or qc == 7:
                            # half of att@V + its normalization, emitted as
                            # soon as the needed attT blocks + denominators
                            # exist: n-half 0 after qc 3, n-half 1 after qc 7
                            nh = 0 if qc == 3 else 1
                            n0h, n1h = nh * 512, nh * 512 + 512
                            qlo, qhi = nh * 4, nh * 4 + 4
                            rcdb = rcd[pr % 2]
                            rb = rcdb[:]
                            APr = type(rb)
                            for i in range(2):
                                nc.gpsimd.dma_start(
                                    APr(
                                        rb.tensor,
                                        rb.offset + i * L + n0h,
                                        [[1, P], [P, 4]],
                                    ),
                                    rc[:, i, qlo:qhi],
                                )
                            rcbc = rcp.tile([P, 512], f32, tag=f"rcbc{nh}")
                            nc.sync.dma_start(
                                rcbc[0:64, :],
                                APr(rb.tensor, rb.offset + n0h, [[0, 64], [1, 512]]),
                            )
                            nc.sync.dma_start(
                                rcbc[64:128, :],
                                APr(rb.tensor, rb.offset + L + n0h, [[0, 64], [1, 512]]),
                            )
                            yp = yps.tile([P, 512], f32, tag="yp")
                            ccs = [c for c in range(TC) if c * P < n1h]
                            for cc in ccs:
                                lo = max(cc * P, n0h)
                                w = n1h - lo
                                nc.tensor.matmul(
                                    yp[0:64, lo - n0h : 512],
                                    v_sb[cc][:, h0 * 64 : h0 * 64 + 64],
                                    attT2[0][:, cc, lo:n1h],
                                    start=(cc == 0),
                                    stop=(cc == ccs[-1]),
                                )
                                nc.tensor.matmul(
                                    yp[64:128, lo - n0h : 512],
                                    v_sb[cc][:, h1 * 64 : h1 * 64 + 64],
                                    attT2[1][:, cc, lo:n1h],
                                    start=(cc == 0),
                                    stop=(cc == ccs[-1]),
                                    tile_position=(0, 64),
                                )
                            nc.vector.tensor_tensor(
                                ysb[pr][:, n0h:n1h], yp[:], rcbc[:], op=ALU.mult
                            )

            qkv_pool.release()

            mark("proj_rs")
            # ---------------- proj (partial) + single bf16 ReduceScatter ----
            with tc.tile_pool(
                name="asb", bufs=1
            ) as asbp, tc.tile_pool(name="aps", bufs=4, space="PSUM") as apsp:
                # dummy sqrt whose output feeds LN2's eps: forces the sqrt
                # table load to happen HERE (ACT idle) instead of on the
                # LN2 critical chain
                eps2 = cst.tile([P, 1], f32)
                nc.scalar.activation(eps2[:], eps_t[:], AF.Sqrt)
                nc.scalar.activation(eps2[:], eps2[:], AF.Square)
                asb = asbp.tile([P, TC, D], bf16)
                for t in range(TC):
                    for n in range(2):
                        ap_ = apsp.tile([P, 512], f32, tag="ap")
                        for p in range(4):
                            nc.tensor.matmul(
                                ap_[:],
                                ysb[p][:, t * P : (t + 1) * P],
                                wproj_sb[p][:, n * 512 : (n + 1) * 512],
                                start=(p == 0),
                                stop=(p == 3),
                            )
                        nc.scalar.activation(
                            asb[:, t, n * 512 : (n + 1) * 512], ap_[:], AF.Copy
                        )
                    # one row-block write per token chunk, fired as soon as
                    # that chunk's projection completes
                    nc.gpsimd.dma_start(
                        cc_in[t * P : (t + 1) * P, :], asb[:, t, :]
                    )
                    if no_rs and t < T2:
                        # stand-in for the RS: copy + readback fired per
                        # chunk so token 0's residual/LN2 chain starts while
                        # later chunks still project
                        nc.sync.dma_start(
                            cc_out[t * P : (t + 1) * P, :],
                            cc_in[t * P : (t + 1) * P, :],
                        )
                        nc.sync.dma_start(
                            ar_all[:, t, :], cc_out[t * P : (t + 1) * P, :]
                        )
                if no_rs:
                    pass
                else:
                    nc.gpsimd.collective_compute(
                        "ReduceScatter",
                        mybir.AluOpType.add,
                        replica_groups=[[0, 1], [2, 3], [4, 5], [6, 7]],
                        ins=[cc_in[:]],
                        outs=[cc_out[:]],
                    )
            ysb_pool.release()

            # fc1's PSUM pool allocated BEFORE the LN2 transpose pool so
            # fc1's first matmuls (which need only LN2 chunks 0-1) don't wait
            # for the transpose pool to close (stack allocator)
            fc1ps = tc.alloc_tile_pool(name="fc1ps", bufs=4, space="PSUM")

            mark("ln2")
            # ---------------- residual + LN2 + h2T ----------------
            x2p = es.enter_context(tc.tile_pool(name="x2p", bufs=1))
            x2 = [x2p.tile([P, D], f32, name=f"x2_{t}") for t in range(T2)]
            h2T = [h2Tp.tile([P, TMY], bf16, name=f"h2T{d}") for d in range(DCH)]
            with tc.tile_pool(name="res", bufs=2) as resp, tc.tile_pool(
                name="lnscr2", bufs=2
            ) as lnscr2:
                h2 = [resp.tile([P, D], bf16, name=f"h2_{t}", bufs=1) for t in range(T2)]
                if not no_rs:
                    nc.sync.dma_start(
                        ar_all[:], cc_out[:].rearrange("(t p) c -> p t c", p=P)
                    )
                for t in range(T2):
                    nc.vector.tensor_tensor(
                        x2[t][:], xm_tiles[t][:], ar_all[:, t, :], op=ALU.add
                    )
                    if use_bproj:
                        nc.vector.tensor_tensor(
                            x2[t][:], x2[t][:], bproj_bc[:], op=ALU.add
                        )
                layernorm(
                    tc, nc, (small, lnscr2), x2, h2, T2, aff2, ln2w_bc, ln2b_bc,
                    eps2[:],
                )
                with tc.tile_pool(name="h2ps", bufs=2, space="PSUM") as h2ps:
                    for t in range(T2):
                        tp = h2ps.tile([P, 4, P], bf16, tag="h2p")
                        tp2 = h2ps.tile([P, 4, P], bf16, tag="h2p2")
                        for d in range(DCH):
                            dst = tp if d < 4 else tp2
                            nc.tensor.transpose(
                                dst[:, d % 4, :], h2[t][:, d * P : (d + 1) * P],
                                id16[:],
                            )
                        for d in range(DCH):
                            dst = tp if d < 4 else tp2
                            nc.vector.tensor_copy(
                                h2T[d][:, t * P : (t + 1) * P], dst[:, d % 4, :]
                            )

            mark("fc1")
            # ---------------- FFN ----------------
            m1p = es.enter_context(tc.tile_pool(name="m1p", bufs=1))
            m1T = [m1p.tile([P, TMY], bf16, name=f"m1T{f}") for f in range(FC)]
            with tc.tile_pool(name="wfcp", bufs=4) as wfcp:
                for half in range(2):
                    # one [128, 2048] weight tile per d-chunk covers 16
                    # f-chunks (4KB contiguous rows -> efficient DMA)
                    wts = []
                    for d in range(DCH):
                        wt = wfcp.tile([P, 2048], bf16, tag=f"wfc{d % 4}")
                        nc.sync.dma_start(
                            wt[:],
                            wfc_in[d * P : (d + 1) * P,
                                   half * 2048 : (half + 1) * 2048],
                        )
                        wts.append(wt)
                    for fl in range(16):
                        f = half * 16 + fl
                        mp = fc1ps.tile([P, TMY], f32, tag="m1ps")
                        # token-halved rhs so the first matmuls only need
                        # LN2 chunks 0-1 (chunks 2-3 may still be in flight)
                        for th in range(2):
                            tsl = slice(th * 256, th * 256 + 256)
                            for d in range(DCH):
                                nc.tensor.matmul(
                                    mp[:, tsl],
                                    wts[d][:, fl * P : (fl + 1) * P],
                                    h2T[d][:, tsl],
                                    start=(d == 0),
                                    stop=(d == DCH - 1),
                                )
                        if use_bfc:
                            nc.scalar.activation(
                                m1T[f][:], mp[:], AF.Gelu, bias=bfc_sb[:, f : f + 1]
                            )
                        else:
                            nc.scalar.activation(m1T[f][:], mp[:], AF.Gelu)

            mark("fc2")
            # fc2 t-major with fully-resident weights (loaded during fc1) so
            # each token chunk's output lands early and its DMA overlaps the
            # remaining matmuls.
            with tc.tile_pool(name="wfc2p", bufs=1) as wfc2p, tc.tile_pool(
                name="outp", bufs=1
            ) as outp, tc.tile_pool(name="fc2ps", bufs=2, space="PSUM") as fc2ps:
                w2 = [wfc2p.tile([P, D], bf16, name=f"w2_{f}") for f in range(FC)]
                for f in range(FC):
                    nc.sync.dma_start(w2[f][:], wfc2_in[f * P : (f + 1) * P, :])
                out_sb = [outp.tile([P, D], f32, name=f"o{t}") for t in range(T2)]
                pss = {}
                for t in range(T2):
                    pss[t] = [
                        fc2ps.tile([P, 512], f32, tag=f"fc2_{n}", name=f"p2_{t}_{n}")
                        for n in range(2)
                    ]
                    for f in range(FC):
                        for n in range(2):
                            nc.tensor.matmul(
                                pss[t][n][:],
                                m1T[f][:, t * P : (t + 1) * P],
                                w2[f][:, n * 512 : (n + 1) * 512],
                                start=(f == 0),
                                stop=(f == FC - 1),
                            )
                    for n in range(2):
                        nc.vector.tensor_tensor(
                            out_sb[t][:, n * 512 : (n + 1) * 512],
                            pss[t][n][:],
                            x2[t][:, n * 512 : (n + 1) * 512],
                            op=ALU.add,
                        )
                    if use_bfc2:
                        nc.vector.tensor_tensor(
                            out_sb[t][:], out_sb[t][:], bfc2_bc[:], op=ALU.add
                        )
                    for n in range(2):
                        nc.sync.dma_start(
                            out_dram[t * P : (t + 1) * P, n * 512 : (n + 1) * 512],
                            out_sb[t][:, n * 512 : (n + 1) * 512],
                        )

            fc1ps.release()

    mark("end")
    nc.compile()
    return nc


def _get_program(flags):
    if flags not in _PROGRAM_CACHE:
        _PROGRAM_CACHE[flags] = _build_program(flags)
    return _PROGRAM_CACHE[flags]


def kernel(
    x,
    ln1_w,
    ln1_b,
    Wqkv,
    bqkv,
    Wproj,
    bproj,
    Er,
    ln2_w,
    ln2_b,
    Wfc,
    bfc,
    Wfc2,
    bfc2,
):
    import ml_dtypes

    from concourse.bass_utils import run_bass_kernel_spmd

    bf = ml_dtypes.bfloat16
    x = np.asarray(x, np.float32)
    f = np.float32
    ntriv = lambda a, v: not np.all(np.asarray(a) == v)
    flags = (
        ntriv(ln1_w, 1) or ntriv(ln1_b, 0),
        ntriv(ln2_w, 1) or ntriv(ln2_b, 0),
        ntriv(bqkv[:D], 0),
        ntriv(bqkv[D : 2 * D], 0),
        ntriv(bqkv[2 * D :], 0),
        ntriv(bproj, 0),
        ntriv(bfc, 0),
        ntriv(bfc2, 0),
    )
    nc = _get_program(flags)

    ErT = np.asarray(Er, f).T * ERPRE           # [HS, L], 8x prescale
    ert2 = np.ascontiguousarray(
        np.concatenate([ErT, ErT], axis=0)
    ).astype(bf)
    c = np.ascontiguousarray
    Wqkv = np.asarray(Wqkv, f)
    Wfc_b = np.asarray(Wfc, f).astype(bf)
    Wfc2_b = np.asarray(Wfc2, f).astype(bf)
    in_maps = []
    for core in range(8):
        b, half = divmod(core, 2)
        hs0, hs1 = half * 512, (half + 1) * 512
        bq = np.asarray(bqkv[:D][hs0:hs1], f) * SCALE
        bk = np.asarray(bqkv[D : 2 * D][hs0:hs1], f)
        wqkv_half = np.concatenate(
            [
                Wqkv[:, 0:D][:, hs0:hs1],
                Wqkv[:, D : 2 * D][:, hs0:hs1],
                Wqkv[:, 2 * D :][:, hs0:hs1],
            ],
            axis=1,
        ).astype(bf)
        in_maps.append(
            {
                "x": c(x[b]).astype(bf),
                "x_my": c(x[b, hs0:hs1], f),
                "wqkv": c(wqkv_half),
                "wproj": c(np.asarray(Wproj, f)[hs0:hs1, :]).astype(bf),
                "ert2": ert2,
                "wfc": Wfc_b,
                "wfc2": Wfc2_b,
                "ln1a": c(np.asarray(ln1_w), f),
                "ln1b": c(np.asarray(ln1_b), f),
                "ln2a": c(np.asarray(ln2_w), f),
                "ln2b": c(np.asarray(ln2_b), f),
                "bq": c(bq.reshape(4, P).T, f),
                "bk": c(bk.reshape(4, P).T, f),
                "bv": c(np.asarray(bqkv[2 * D :][hs0:hs1]), f),
                "bproj": c(np.asarray(bproj), f),
                "bfc": c(np.asarray(bfc).reshape(FC, P).T, f),
                "bfc2": c(np.asarray(bfc2), f),
            }
        )

    trace = bool(int(os.environ.get("KERNEL_TRACE", "0")))
    res = run_bass_kernel_spmd(nc, in_maps, list(range(8)), trace=trace)
    global LAST_EXEC_NS, LAST_RESULT
    LAST_EXEC_NS = res.exec_time_ns
    LAST_RESULT = res
    out = np.empty((B, L, D), np.float32)
    for core in range(8):
        b, half = divmod(core, 2)
        out[b, half * 512 : (half + 1) * 512] = res.results[core]["out_my"]
    return out


LAST_EXEC_NS = None
LAST_RESULT = None


# revision 89
# speedup vs baseline: 1.0455x; 1.0265x over previous
"""Trainium2 Bass kernel for nn_BlockWithCache (Music-Transformer block w/ rel-pos).

Sharding (8 NeuronCores, uniform SPMD program; per-core differences live in the
input data only):
  - core c: batch element b = c//2, tensor-parallel half = c%2.
  - Attention: TP over heads — each core computes its 8 of 16 heads for the
    full 1024-token sequence (weight column slices supplied by the host).
  - Wproj row-slices produce partial attention outputs; a single pairwise
    bf16 ReduceScatter(add) over [L, D] both completes the sum and splits
    tokens in half.
  - From the residual on: token-split — each core owns 512 tokens through
    LN2 + FFN (full 4*D hidden) and writes a disjoint output half.

Key tricks:
  - bf16 operands everywhere on the matmul paths (weights cast on host),
    fp32 PSUM accumulation; halves weight DMA vs fp32.
  - Music-Transformer skew: QEr rows round-trip through a DRAM buffer (both
    heads interleaved per row so write+read are single DMAs) written with row
    stride 2*srow and read back with row stride 2*srow-1, which realigns
    QEr[q, 1023-q+c] to [q, c]; the 128-wide pad region holds -240 (fp8e4 max
    FINITE -- the format has inf, and an -inf pad would make the 0*(-inf)
    matmul terms NaN) so the causal mask comes back for free (exp -> ~0).
    The buffer is fp8e4 with an 8x prescale folded into Er host-side and
    un-done by a 0.125-diagonal in the Srel-add identity matmul.
  - Softmax without max-subtraction (logits are small), denominator via the
    ACT engine's fused accum_out. The exp tiles stay UNNORMALIZED; the
    1/denominator is applied once at the y = att@V read-out, with the
    per-token reciprocal row replicated across partitions by a stride-0
    partition-broadcast DMA read from DRAM. att@V runs per 512-column half,
    emitted as soon as that half's attT blocks + denominators exist.
  - attT via PE transpose (bf16), copies batched 4 blocks per DVE op; R-phase
    PSUM evacuations split DVE/ACT to balance the two busiest engines.
  - One bf16 ReduceScatter over the full [L, D] proj partials (replaces two
    fp32 collectives); per-token-chunk writes let token 0's residual + LN2
    chain start while later chunks still project.
"""

import os
import sys

os.environ.setdefault("MYCRO_LOCAL_CACHE", "1")
if "/opt/trn_rl_repo" not in sys.path:
    sys.path.insert(0, "/opt/trn_rl_repo")

import numpy as np

B, L, D, H = 4, 1024, 1024, 16
HS = D // H          # 64
P = 128
TC = L // P          # 8 token chunks
DCH = D // P         # 8 feature chunks
NHC = H // 2         # 8 heads per core
FD = 4 * D           # 4096
FC = FD // P         # 32
TMY = L // 2         # 512 tokens owned after RS
T2 = TMY // P        # 4
EPS = 1e-5
SCALE = 1.0 / 8.0    # 1/sqrt(HS)
ERPRE = 8.0          # fp8 skew prescale (undone by 0.125 diag)
NEGF8 = -240.0       # fp8e4 max finite (fmt has inf!); -240*0.125=-30 after
                     # diag -> exp(-30+qk) ~ 1e-11 ~ 0. Must stay finite:
                     # an -inf pad would make the 0*(-inf) matmul terms NaN.
_PROGRAM_CACHE = {}
PHASE_MARKS = []


def _build_program(flags, no_rs=False):
    import concourse.mybir as mybir
    import concourse.tile as tile
    from concourse import bacc
    from concourse.masks import make_identity

    (aff1, aff2, use_bq, use_bk, use_bv, use_bproj, use_bfc, use_bfc2) = flags

    f32 = mybir.dt.float32
    bf16 = mybir.dt.bfloat16
    fp8 = mybir.dt.float8e4
    AF = mybir.ActivationFunctionType
    ALU = mybir.AluOpType
    AX = mybir.AxisListType

    nc = bacc.Bacc("TRN2", target_bir_lowering=False, debug=False, num_devices=8)
    PHASE_MARKS.clear()

    def mark(label):
        PHASE_MARKS.append((label, nc.next_id()))

    x_in = nc.declare_dram_parameter("x", [L, D], bf16, isOutput=False)
    xmy_in = nc.declare_dram_parameter("x_my", [TMY, D], f32, isOutput=False)
    wqkv_in = nc.declare_dram_parameter("wqkv", [D, 3 * NHC * HS], bf16, isOutput=False)
    wproj_in = nc.declare_dram_parameter("wproj", [NHC * HS, D], bf16, isOutput=False)
    ert2_in = nc.declare_dram_parameter("ert2", [P, L], bf16, isOutput=False)
    wfc_in = nc.declare_dram_parameter("wfc", [D, FD], bf16, isOutput=False)
    wfc2_in = nc.declare_dram_parameter("wfc2", [FD, D], bf16, isOutput=False)
    # Always-declared small params (cheap; used only when flags set)
    ln1a_in = nc.declare_dram_parameter("ln1a", [D], f32, isOutput=False)
    ln1b_in = nc.declare_dram_parameter("ln1b", [D], f32, isOutput=False)
    ln2a_in = nc.declare_dram_parameter("ln2a", [D], f32, isOutput=False)
    ln2b_in = nc.declare_dram_parameter("ln2b", [D], f32, isOutput=False)
    bq_in = nc.declare_dram_parameter("bq", [P, 4], f32, isOutput=False)
    bk_in = nc.declare_dram_parameter("bk", [P, 4], f32, isOutput=False)
    bv_in = nc.declare_dram_parameter("bv", [NHC * HS], f32, isOutput=False)
    bproj_in = nc.declare_dram_parameter("bproj", [D], f32, isOutput=False)
    bfc_in = nc.declare_dram_parameter("bfc", [P, FC], f32, isOutput=False)
    bfc2_in = nc.declare_dram_parameter("bfc2", [D], f32, isOutput=False)

    out_dram = nc.declare_dram_parameter("out_my", [TMY, D], f32, isOutput=True)

    def layernorm(tc, nc, pools, xs, hs, nchunks, aff, wbc, bbc, eps_ap):
        """Per-chunk two-pass LN (bf16 in, bf16 out) so chunk t's output is
        ready without waiting on later chunks."""
        small, scratch = pools
        for t in range(nchunks):
            st = small.tile([P, 8], f32, tag="ln_st")
            # st cols: 0 sum, 1 sumsq, 2 mu, 3 mu^2, 4 var, 5 std, 6 rstd, 7 mur
            nc.vector.reduce_sum(st[:, 0:1], xs[t][:], axis=AX.X)
            sq = scratch.tile([P, D], f32, tag="ln_sq")
            nc.scalar.activation(sq[:], xs[t][:], AF.Square, accum_out=st[:, 1:2])
            # fused: mu^2 = (sum/D^2)*sum; mu*rstd = (sum/D)*rstd
            nc.vector.scalar_tensor_tensor(
                st[:, 3:4], st[:, 0:1], 1.0 / (D * D), st[:, 0:1],
                op0=ALU.mult, op1=ALU.mult,
            )
            nc.vector.tensor_scalar(
                st[:, 4:5], st[:, 1:2], 1.0 / D, st[:, 3:4],
                op0=ALU.mult, op1=ALU.subtract,
            )
            nc.scalar.activation(st[:, 5:6], st[:, 4:5], AF.Sqrt, bias=eps_ap)
            nc.vector.reciprocal(st[:, 6:7], st[:, 5:6])
            nc.vector.scalar_tensor_tensor(
                st[:, 7:8], st[:, 0:1], 1.0 / D, st[:, 6:7],
                op0=ALU.mult, op1=ALU.mult,
            )
            if aff:
                hf = scratch.tile([P, D], f32, tag="ln_hf")
                nc.vector.tensor_scalar(
                    hf[:], xs[t][:], st[:, 6:7], st[:, 7:8],
                    op0=ALU.mult, op1=ALU.subtract,
                )
                nc.vector.tensor_tensor(hf[:], hf[:], wbc[:], op=ALU.mult)
                nc.vector.tensor_tensor(hs[t][:], hf[:], bbc[:], op=ALU.add)
            else:
                nc.vector.tensor_scalar(
                    hs[t][:], xs[t][:], st[:, 6:7], st[:, 7:8],
                    op0=ALU.mult, op1=ALU.subtract,
                )

    with tile.TileContext(nc) as tc:
        import contextlib

        with contextlib.ExitStack() as es:
            cst = es.enter_context(tc.tile_pool(name="cst", bufs=1))
            small = es.enter_context(tc.tile_pool(name="small", bufs=2))
            dram = es.enter_context(tc.tile_pool(name="dram", bufs=1, space="DRAM"))

            h2Tp = es.enter_context(tc.tile_pool(name="h2Tp", bufs=1))

            # x chunks + residual rows first: these DMAs gate the LN1 pipeline
            # and the HWDGE queue is strictly FIFO per issue order.
            xmp = es.enter_context(tc.tile_pool(name="xmp", bufs=1))
            ysb_pool = tc.alloc_tile_pool(name="ysb", bufs=1)
            qkv_pool = tc.alloc_tile_pool(name="qkv", bufs=1)
            xph = tc.alloc_tile_pool(name="xp", bufs=1)
            xs_tiles = [xph.tile([P, D], bf16, name=f"x{t}") for t in range(TC)]
            for t in range(TC):
                nc.sync.dma_start(xs_tiles[t][:], x_in[t * P : (t + 1) * P, :])
            xm_tiles = [xmp.tile([P, D], f32, name=f"xm{t}") for t in range(T2)]
            for t in range(T2):
                nc.sync.dma_start(xm_tiles[t][:], xmy_in[t * P : (t + 1) * P, :])
            ar_all = xmp.tile([P, T2, D], bf16)
            wproj_sb = [xmp.tile([P, D], bf16, name=f"wpj{p}") for p in range(4)]
            for p in range(4):
                nc.sync.dma_start(wproj_sb[p][:], wproj_in[p * P : (p + 1) * P, :])

            eps_t = cst.tile([P, 1], f32)
            nc.vector.memset(eps_t[:], EPS)
            id16 = cst.tile([P, P], bf16)
            make_identity(nc, id16)
            # fp8 identity with 0.125 diagonal (undoes the 8x Er prescale)
            id8 = cst.tile([P, P], fp8)
            nc.gpsimd.memset(id8[:], 0.0)
            nc.gpsimd.affine_select(
                out=id8[:],
                in_=id8[:],
                compare_op=mybir.AluOpType.not_equal,
                fill=0.125,
                base=0,
                pattern=[[-1, P]],
                channel_multiplier=1,
            )
            ert2 = cst.tile([P, L], bf16)
            nc.sync.dma_start(ert2[:], ert2_in[:])

            ln1w_bc = ln1b_bc = ln2w_bc = ln2b_bc = None
            if aff1:
                row = cst.tile([1, D], f32, tag="lnrow1a")
                nc.sync.dma_start(row[:], ln1a_in[None, :])
                ln1w_bc = cst.tile([P, D], f32)
                nc.gpsimd.partition_broadcast(ln1w_bc[:], row[:])
                row2 = cst.tile([1, D], f32, tag="lnrow1b")
                nc.sync.dma_start(row2[:], ln1b_in[None, :])
                ln1b_bc = cst.tile([P, D], f32)
                nc.gpsimd.partition_broadcast(ln1b_bc[:], row2[:])
            if aff2:
                row = cst.tile([1, D], f32, tag="lnrow2a")
                nc.sync.dma_start(row[:], ln2a_in[None, :])
                ln2w_bc = cst.tile([P, D], f32)
                nc.gpsimd.partition_broadcast(ln2w_bc[:], row[:])
                row2 = cst.tile([1, D], f32, tag="lnrow2b")
                nc.sync.dma_start(row2[:], ln2b_in[None, :])
                ln2b_bc = cst.tile([P, D], f32)
                nc.gpsimd.partition_broadcast(ln2b_bc[:], row2[:])
            bq_sb = bk_sb = None
            if use_bq:
                bq_sb = cst.tile([P, 4], f32)
                nc.sync.dma_start(bq_sb[:], bq_in[:])
            if use_bk:
                bk_sb = cst.tile([P, 4], f32)
                nc.sync.dma_start(bk_sb[:], bk_in[:])
            bv_bc = None
            if use_bv:
                row = cst.tile([1, NHC * HS], f32, tag="bvrow")
                nc.sync.dma_start(row[:], bv_in[None, :])
                bv_bc = cst.tile([P, NHC * HS], f32)
                nc.gpsimd.partition_broadcast(bv_bc[:], row[:])
            bproj_bc = None
            if use_bproj:
                row = cst.tile([1, D], f32, tag="bprow")
                nc.sync.dma_start(row[:], bproj_in[None, :])
                bproj_bc = cst.tile([P, D], f32)
                nc.gpsimd.partition_broadcast(bproj_bc[:], row[:])
            bfc_sb = None
            if use_bfc:
                bfc_sb = cst.tile([P, FC], f32)
                nc.sync.dma_start(bfc_sb[:], bfc_in[:])
            bfc2_bc = None
            if use_bfc2:
                row = cst.tile([1, D], f32, tag="b2row")
                nc.sync.dma_start(row[:], bfc2_in[None, :])
                bfc2_bc = cst.tile([P, D], f32)
                nc.gpsimd.partition_broadcast(bfc2_bc[:], row[:])

            # Proj-partial DRAM buffers for the single bf16 ReduceScatter
            cc_in = dram.tile([L, D], bf16, name="cc_in")
            cc_out = dram.tile([TMY, D], bf16, name="cc_out")
            # reciprocal-row round-trip buffers (one per head-pair parity)
            rcd = [dram.tile([2, L], f32, name=f"rcd{n}") for n in range(2)]

            # Skew DRAM buffers: per q-chunk, 2 slots (fp8), both heads
            # interleaved per row so write+read are single DMAs.
            # Layout: elem (q-row r, head i, col c) at r*2*srow + i*srow + c.
            negpad = cst.tile([P, 2, P], fp8)
            nc.vector.memset(negpad[:], NEGF8)
            skewbufs = []
            for qc in range(TC):
                srow = P * (qc + 2)
                wm = P * (qc + 1)
                slots = []
                for s in range(2):
                    d2 = dram.tile([2 * P * srow], fp8, name=f"skew_{qc}_{s}")
                    wv_full = d2[:].rearrange("(r i c) -> r i c", i=2, c=srow)
                    nc.gpsimd.dma_start(wv_full[:, :, wm:], negpad[:])
                    slots.append(d2)
                skewbufs.append(slots)

            # ---------------- persistent activation tiles ----------------
            ysb = [ysb_pool.tile([P, L], bf16, name=f"ysb{p}") for p in range(4)]

            qt_sb = [qkv_pool.tile([P, L], bf16, name=f"qt{p}") for p in range(4)]
            kt_sb = [qkv_pool.tile([P, L], bf16, name=f"kt{p}") for p in range(4)]
            v_sb = [qkv_pool.tile([P, NHC * HS], bf16, name=f"v{t}") for t in range(TC)]

            mark("ln1")
            # ---------------- LN1 + transpose + QKV ----------------
            with tc.tile_pool(name="hT", bufs=1) as hTp:
                hT = [hTp.tile([P, L], bf16, name=f"hT{d}") for d in range(DCH)]
                with tc.tile_pool(name="xh", bufs=1) as xh, tc.tile_pool(
                    name="lnscr", bufs=2
                ) as lnscr:
                    xs = xs_tiles
                    hs = [xh.tile([P, D], bf16, name=f"h{t}") for t in range(TC)]
                    layernorm(
                        tc, nc, (small, lnscr), xs, hs, TC, aff1, ln1w_bc, ln1b_bc,
                        eps_t[:],
                    )
                    with tc.tile_pool(name="htps", bufs=4, space="PSUM") as htps:
                        for t in range(TC):
                            tp = htps.tile([P, 4, P], bf16, tag="htp")
                            tp2 = htps.tile([P, 4, P], bf16, tag="htp2")
                            for d in range(DCH):
                                dst = tp if d < 4 else tp2
                                nc.tensor.transpose(
                                    dst[:, d % 4, :], hs[t][:, d * P : (d + 1) * P],
                                    id16[:],
                                )
                            for d in range(DCH):
                                dst = tp if d < 4 else tp2
                                nc.vector.tensor_copy(
                                    hT[d][:, t * P : (t + 1) * P], dst[:, d % 4, :]
                                )

                # QKV projections (h freed; hT alive)
                with tc.tile_pool(name="wqkv", bufs=1) as wp, tc.tile_pool(
                    name="qkvps", bufs=4, space="PSUM"
                ) as qps:
                    wqkv_sb = [
                        wp.tile([P, 3 * NHC * HS], bf16, name=f"wqkv{d}")
                        for d in range(DCH)
                    ]
                    for d in range(DCH):
                        nc.sync.dma_start(
                            wqkv_sb[d][:], wqkv_in[d * P : (d + 1) * P, :]
                        )
                    # Q^T and K^T: out [128(2 heads), tokens]
                    for p in range(4):
                        for n in range(2):
                            ps = qps.tile([P, 512], f32, tag="qkvp")
                            for d in range(DCH):
                                nc.tensor.matmul(
                                    ps[:],
                                    wqkv_sb[d][:, p * P : (p + 1) * P],
                                    hT[d][:, n * 512 : (n + 1) * 512],
                                    start=(d == 0),
                                    stop=(d == DCH - 1),
                                )
                            nc.scalar.activation(
                                qt_sb[p][:, n * 512 : (n + 1) * 512],
                                ps[:],
                                AF.Copy,
                                scale=SCALE,
                            )
                            if use_bq:
                                nc.vector.tensor_scalar_add(
                                    qt_sb[p][:, n * 512 : (n + 1) * 512],
                                    qt_sb[p][:, n * 512 : (n + 1) * 512],
                                    bq_sb[:, p : p + 1],
                                )
                        for n in range(2):
                            ps = qps.tile([P, 512], f32, tag="qkvp")
                            for d in range(DCH):
                                nc.tensor.matmul(
                                    ps[:],
                                    wqkv_sb[d][:, 512 + p * P : 512 + (p + 1) * P],
                                    hT[d][:, n * 512 : (n + 1) * 512],
                                    start=(d == 0),
                                    stop=(d == DCH - 1),
                                )
                            nc.scalar.activation(
                                kt_sb[p][:, n * 512 : (n + 1) * 512], ps[:], AF.Copy
                            )
                            if use_bk:
                                nc.vector.tensor_scalar_add(
                                    kt_sb[p][:, n * 512 : (n + 1) * 512],
                                    kt_sb[p][:, n * 512 : (n + 1) * 512],
                                    bk_sb[:, p : p + 1],
                                )
                    # V: out [tokens, 512 hs-cols]
                    for t in range(TC):
                        ps = qps.tile([P, 512], f32, tag="qkvp")
                        for d in range(DCH):
                            nc.tensor.matmul(
                                ps[:],
                                hT[d][:, t * P : (t + 1) * P],
                                wqkv_sb[d][:, 1024:1536],
                                start=(d == 0),
                                stop=(d == DCH - 1),
                            )
                        if use_bv:
                            nc.vector.tensor_tensor(
                                ps[:], ps[:], bv_bc[:], op=ALU.add
                            )
                        nc.scalar.activation(v_sb[t][:], ps[:], AF.Copy)

            xph.release()

            mark("attention")
            # ---------------- attention ----------------
            with contextlib.ExitStack() as att_es:
                expp = att_es.enter_context(tc.tile_pool(name="expp", bufs=8))
                srelp = att_es.enter_context(tc.tile_pool(name="srelp", bufs=4))
                rsbp = att_es.enter_context(tc.tile_pool(name="rsbp", bufs=6))
                attTp = att_es.enter_context(tc.tile_pool(name="attTp", bufs=3))
                dnp = att_es.enter_context(tc.tile_pool(name="dnp", bufs=4))
                rcp = att_es.enter_context(tc.tile_pool(name="rcp", bufs=2))
                sps = att_es.enter_context(tc.tile_pool(name="sps", bufs=3, space="PSUM"))
                rps = att_es.enter_context(tc.tile_pool(name="rps", bufs=1, space="PSUM"))
                tps = att_es.enter_context(tc.tile_pool(name="tps", bufs=2, space="PSUM"))
                yps = att_es.enter_context(tc.tile_pool(name="yps", bufs=1, space="PSUM"))

                def emit_rphase(pr):
                    """R = Q Er^T (both heads, concurrent row groups) -> DRAM
                    skew write -> skewed read (Srel, fp8). Both heads share
                    one interleaved buffer so write+read are single DMAs."""
                    srels = []
                    for qc in range(TC):
                        wp_ = P * (qc + 1)
                        m0 = 896 - P * qc
                        srow = P * (qc + 2)
                        nsub = (wp_ + 511) // 512
                        d2 = skewbufs[qc][pr % 2]
                        base = d2[:]
                        APc = type(base)
                        wview = base.rearrange("(r i c) -> r i c", i=2, c=srow)
                        rview = APc(
                            base.tensor,
                            base.offset + 127,
                            [[2 * srow - 1, P], [srow, 2], [1, wp_]],
                        )
                        rsb = rsbp.tile([P, 2, wp_], fp8, tag="rsb")
                        for i in range(2):
                            off = i * 64
                            lhsq = qt_sb[pr][off : off + 64, qc * P : (qc + 1) * P]
                            for s in range(nsub):
                                w = min(512, wp_ - s * 512)
                                rp = rps.tile([P, 512], f32, tag=f"rp{i}")
                                nc.tensor.matmul(
                                    rp[:, :w],
                                    lhsq,
                                    ert2[off : off + 64, m0 + s * 512 : m0 + s * 512 + w],
                                    start=True,
                                    stop=True,
                                )
                                # engine balance: DVE is the busiest engine
                                # in the attention window, so shift a slice
                                # of the PSUM->SBUF evacuations to ACT
                                if i == 1 and qc >= 6:
                                    nc.scalar.activation(
                                        rsb[:, i, s * 512 : s * 512 + w],
                                        rp[:, :w],
                                        AF.Copy,
                                    )
                                else:
                                    nc.vector.tensor_copy(
                                        rsb[:, i, s * 512 : s * 512 + w], rp[:, :w]
                                    )
                        nc.sync.dma_start(wview[:, :, :wp_], rsb[:])
                        srel = srelp.tile([P, 2, wp_], fp8, tag=f"srel{qc}")
                        nc.sync.dma_start(srel[:], rview)
                        srels.append(srel)
                    return srels

                srel_pending = {0: emit_rphase(0)}
                for pr in range(4):
                    h0, h1 = 2 * pr, 2 * pr + 1
                    if pr + 1 < 4:
                        srel_pending[pr + 1] = emit_rphase(pr + 1)
                    srels2 = srel_pending.pop(pr)
                    attT2 = [
                        attTp.tile([P, TC, L], bf16, tag="attT", name=f"attT_{pr}_{i}")
                        for i in range(2)
                    ]
                    dn = dnp.tile([P, 2, TC, 2], f32, tag="dn")
                    dns = dnp.tile([P, 2, TC], f32, tag="dns")
                    rc = dnp.tile([P, 2, TC], f32, tag="rc")
                    for qc in range(TC):
                        wp_ = P * (qc + 1)     # W' = causal width
                        nsub = (wp_ + 511) // 512
                        lhsq2 = [
                            qt_sb[pr][0:64, qc * P : (qc + 1) * P],
                            qt_sb[pr][64:128, qc * P : (qc + 1) * P],
                        ]
                        exp2 = [
                            expp.tile([P, wp_], bf16, tag="exp", name=f"ex_{pr}_{qc}_{i}")
                            for i in range(2)
                        ]
                        for s in range(nsub):
                            w = min(512, wp_ - s * 512)
                            sl = slice(s * 512, s * 512 + w)
                            sp2 = [
                                sps.tile([P, 512], f32, tag="sp", name=f"sp_{qc}_{s}_{i}")
                                for i in range(2)
                            ]
                            # the two heads' QK matmuls use disjoint PE row
                            # groups (K rows 0-63 vs 64-127) -> run concurrent
                            for i in range(2):
                                nc.tensor.matmul(
                                    sp2[i][:, :w],
                                    lhsq2[i],
                                    kt_sb[pr][64 * i : 64 * i + 64, sl],
                                    start=True,
                                    stop=False,
                                )
                            # += Srel (with fp8-min causal pad) via 0.125-diag
                            # identity matmul (undoes the 8x Er prescale)
                            for i in range(2):
                                nc.tensor.matmul(
                                    sp2[i][:, :w],
                                    id8[:],
                                    srels2[qc][:, i, sl],
                                    start=False,
                                    stop=True,
                                )
                            for i in range(2):
                                nc.scalar.activation(
                                    exp2[i][:, sl], sp2[i][:, :w], AF.Exp,
                                    accum_out=dn[:, i, qc, s : s + 1],
                                )
                        for i in range(2):
                            if nsub == 2:
                                nc.vector.tensor_tensor(
                                    dns[:, i, qc : qc + 1],
                                    dn[:, i, qc, 0:1],
                                    dn[:, i, qc, 1:2],
                                    op=ALU.add,
                                )
                            else:
                                nc.vector.tensor_copy(
                                    dns[:, i, qc : qc + 1], dn[:, i, qc, 0:1]
                                )
                            nc.vector.reciprocal(
                                rc[:, i, qc : qc + 1], dns[:, i, qc : qc + 1]
                            )
                        # transpose blocks into attT (batched 4-per-copy)
                        for i in range(2):
                            for c0 in range(0, qc + 1, 4):
                                ncc = min(4, qc + 1 - c0)
                                tp4 = tps.tile([P, 4, P], bf16, tag="tp4")
                                for j in range(ncc):
                                    nc.tensor.transpose(
                                        tp4[:, j, :],
                                        exp2[i][:, (c0 + j) * P : (c0 + j + 1) * P],
                                        id16[:],
                                    )
                                nc.vector.tensor_copy(
                                    attT2[i][:, c0 : c0 + ncc, qc * P : (qc + 1) * P],
                                    tp4[:, 0:ncc, :],
                                )
                        if qc == 3 or qc == 7:
                            # half of att@V + its normalization, emitted as
                            # soon as the needed attT blocks + denominators
                            # exist: n-half 0 after qc 3, n-half 1 after qc 7
                            nh = 0 if qc == 3 else 1
                            n0h, n1h = nh * 512, nh * 512 + 512
                            qlo, qhi = nh * 4, nh * 4 + 4
                            rcdb = rcd[pr % 2]
                            rb = rcdb[:]
                            APr = type(rb)
                            for i in range(2):
                                nc.gpsimd.dma_start(
                                    APr(
                                        rb.tensor,
                                        rb.offset + i * L + n0h,
                                        [[1, P], [P, 4]],
                                    ),
                                    rc[:, i, qlo:qhi],
                                )
                            rcbc = rcp.tile([P, 512], f32, tag=f"rcbc{nh}")
                            nc.sync.dma_start(
                                rcbc[0:64, :],
                                APr(rb.tensor, rb.offset + n0h, [[0, 64], [1, 512]]),
                            )
                            nc.sync.dma_start(
                                rcbc[64:128, :],
                                APr(rb.tensor, rb.offset + L + n0h, [[0, 64], [1, 512]]),
                            )
                            yp = yps.tile([P, 512], f32, tag="yp")
                            ccs = [c for c in range(TC) if c * P < n1h]
                            for cc in ccs:
                                lo = max(cc * P, n0h)
                                w = n1h - lo
                                nc.tensor.matmul(
                                    yp[0:64, lo - n0h : 512],
                                    v_sb[cc][:, h0 * 64 : h0 * 64 + 64],
                                    attT2[0][:, cc, lo:n1h],
                                    start=(cc == 0),
                                    stop=(cc == ccs[-1]),
                                )
                                nc.tensor.matmul(
                                    yp[64:128, lo - n0h : 512],
                                    v_sb[cc][:, h1 * 64 : h1 * 64 + 64],
                                    attT2[1][:, cc, lo:n1h],
                                    start=(cc == 0),
                                    stop=(cc == ccs[-1]),
                                    tile_position=(0, 64),
                                )
                            nc.vector.tensor_tensor(
                                ysb[pr][:, n0h:n1h], yp[:], rcbc[:], op=ALU.mult
                            )

            qkv_pool.release()

            mark("proj_rs")
            # ---------------- proj (partial) + single bf16 ReduceScatter ----
            with tc.tile_pool(
                name="asb", bufs=1
            ) as asbp, tc.tile_pool(name="aps", bufs=4, space="PSUM") as apsp:
                # dummy sqrt whose output feeds LN2's eps: forces the sqrt
                # table load to happen HERE (ACT idle) instead of on the
                # LN2 critical chain
                eps2 = cst.tile([P, 1], f32)
                nc.scalar.activation(eps2[:], eps_t[:], AF.Sqrt)
                nc.scalar.activation(eps2[:], eps2[:], AF.Square)
                asb = asbp.tile([P, TC, D], bf16)
                for t in range(TC):
                    for n in range(2):
                        ap_ = apsp.tile([P, 512], f32, tag="ap")
                        for p in range(4):
                            nc.tensor.matmul(
                                ap_[:],
                                ysb[p][:, t * P : (t + 1) * P],
                                wproj_sb[p][:, n * 512 : (n + 1) * 512],
                                start=(p == 0),
                                stop=(p == 3),
                            )
                        nc.scalar.activation(
                            asb[:, t, n * 512 : (n + 1) * 512], ap_[:], AF.Copy
                        )
                    # one row-block write per token chunk, fired as soon as
                    # that chunk's projection completes
                    nc.gpsimd.dma_start(
                        cc_in[t * P : (t + 1) * P, :], asb[:, t, :]
                    )
                    if no_rs and t < T2:
                        # stand-in for the RS: copy + readback fired per
                        # chunk so token 0's residual/LN2 chain starts while
                        # later chunks still project
                        nc.sync.dma_start(
                            cc_out[t * P : (t + 1) * P, :],
                            cc_in[t * P : (t + 1) * P, :],
                        )
                        nc.sync.dma_start(
                            ar_all[:, t, :], cc_out[t * P : (t + 1) * P, :]
                        )
                if no_rs:
                    pass
                else:
                    nc.gpsimd.collective_compute(
                        "ReduceScatter",
                        mybir.AluOpType.add,
                        replica_groups=[[0, 1], [2, 3], [4, 5], [6, 7]],
                        ins=[cc_in[:]],
                        outs=[cc_out[:]],
                    )
            ysb_pool.release()

            # fc1's PSUM pool allocated BEFORE the LN2 transpose pool so
            # fc1's first matmuls (which need only LN2 chunks 0-1) don't wait
            # for the transpose pool to close (stack allocator)
            fc1ps = tc.alloc_tile_pool(name="fc1ps", bufs=4, space="PSUM")

            mark("ln2")
            # ---------------- residual + LN2 + h2T ----------------
            x2p = es.enter_context(tc.tile_pool(name="x2p", bufs=1))
            x2 = [x2p.tile([P, D], f32, name=f"x2_{t}") for t in range(T2)]
            h2T = [h2Tp.tile([P, TMY], bf16, name=f"h2T{d}") for d in range(DCH)]
            with tc.tile_pool(name="res", bufs=2) as resp, tc.tile_pool(
                name="lnscr2", bufs=2
            ) as lnscr2:
                h2 = [resp.tile([P, D], bf16, name=f"h2_{t}", bufs=1) for t in range(T2)]
                if not no_rs:
                    nc.sync.dma_start(
                        ar_all[:], cc_out[:].rearrange("(t p) c -> p t c", p=P)
                    )
                for t in range(T2):
                    nc.vector.tensor_tensor(
                        x2[t][:], xm_tiles[t][:], ar_all[:, t, :], op=ALU.add
                    )
                    if use_bproj:
                        nc.vector.tensor_tensor(
                            x2[t][:], x2[t][:], bproj_bc[:], op=ALU.add
                        )
                layernorm(
                    tc, nc, (small, lnscr2), x2, h2, T2, aff2, ln2w_bc, ln2b_bc,
                    eps2[:],
                )
                with tc.tile_pool(name="h2ps", bufs=2, space="PSUM") as h2ps:
                    for t in range(T2):
                        tp = h2ps.tile([P, 4, P], bf16, tag="h2p")
                        tp2 = h2ps.tile([P, 4, P], bf16, tag="h2p2")
                        for d in range(DCH):
                            dst = tp if d < 4 else tp2
                            nc.tensor.transpose(
                                dst[:, d % 4, :], h2[t][:, d * P : (d + 1) * P],
                                id16[:],
                            )
                        for d in range(DCH):
                            dst = tp if d < 4 else tp2
                            nc.vector.tensor_copy(
                                h2T[d][:, t * P : (t + 1) * P], dst[:, d % 4, :]
                            )

            mark("fc1")
            # ---------------- FFN ----------------
            m1p = es.enter_context(tc.tile_pool(name="m1p", bufs=1))
            m1T = [m1p.tile([P, TMY], bf16, name=f"m1T{f}") for f in range(FC)]
            # fc2 weights pool allocated BELOW fc1's weight pool in the
            # SBUF stack so the w2 prefetch runs during fc1 (emitted after
            # fc1's own loads to keep priority below the residual chain)
            wfc2p = tc.alloc_tile_pool(name="wfc2p", bufs=1)
            w2 = [wfc2p.tile([P, D], bf16, name=f"w2_{f}") for f in range(FC)]
            with tc.tile_pool(name="wfcp", bufs=4) as wfcp:
                for quarter in range(4):
                    # one [128, 1024] weight tile per d-chunk covers 8
                    # f-chunks (2KB contiguous rows -> efficient DMA)
                    wts = []
                    for d in range(DCH):
                        wt = wfcp.tile([P, 1024], bf16, tag=f"wfc{d % 4}")
                        nc.sync.dma_start(
                            wt[:],
                            wfc_in[d * P : (d + 1) * P,
                                   quarter * 1024 : (quarter + 1) * 1024],
                        )
                        wts.append(wt)
                    for fl in range(8):
                        f = quarter * 8 + fl
                        mp = fc1ps.tile([P, TMY], f32, tag="m1ps")
                        # token-halved rhs so the first matmuls only need
                        # LN2 chunks 0-1 (chunks 2-3 may still be in flight)
                        for th in range(2):
                            tsl = slice(th * 256, th * 256 + 256)
                            for d in range(DCH):
                                nc.tensor.matmul(
                                    mp[:, tsl],
                                    wts[d][:, fl * P : (fl + 1) * P],
                                    h2T[d][:, tsl],
                                    start=(d == 0),
                                    stop=(d == DCH - 1),
                                )
                        if use_bfc:
                            nc.scalar.activation(
                                m1T[f][:], mp[:], AF.Gelu, bias=bfc_sb[:, f : f + 1]
                            )
                        else:
                            nc.scalar.activation(m1T[f][:], mp[:], AF.Gelu)

            for f in range(FC):
                nc.sync.dma_start(w2[f][:], wfc2_in[f * P : (f + 1) * P, :])

            mark("fc2")
            # fc2 t-major with fully-resident weights (prefetched during fc1)
            # so each token chunk's output lands early and its DMA overlaps
            # the remaining matmuls.
            with tc.tile_pool(
                name="outp", bufs=1
            ) as outp, tc.tile_pool(name="fc2ps", bufs=2, space="PSUM") as fc2ps:
                out_sb = [outp.tile([P, D], f32, name=f"o{t}") for t in range(T2)]
                pss = {}
                for t in range(T2):
                    pss[t] = [
                        fc2ps.tile([P, 512], f32, tag=f"fc2_{n}", name=f"p2_{t}_{n}")
                        for n in range(2)
                    ]
                    for f in range(FC):
                        for n in range(2):
                            nc.tensor.matmul(
                                pss[t][n][:],
                                m1T[f][:, t * P : (t + 1) * P],
                                w2[f][:, n * 512 : (n + 1) * 512],
                                start=(f == 0),
                                stop=(f == FC - 1),
                            )
                    for n in range(2):
                        nc.vector.tensor_tensor(
                            out_sb[t][:, n * 512 : (n + 1) * 512],
                            pss[t][n][:],
                            x2[t][:, n * 512 : (n + 1) * 512],
                            op=ALU.add,
                        )
                    if use_bfc2:
                        nc.vector.tensor_tensor(
                            out_sb[t][:], out_sb[t][:], bfc2_bc[:], op=ALU.add
                        )
                    for n in range(2):
                        nc.sync.dma_start(
                            out_dram[t * P : (t + 1) * P, n * 512 : (n + 1) * 512],
                            out_sb[t][:, n * 512 : (n + 1) * 512],
                        )

            wfc2p.release()
            fc1ps.release()

    mark("end")
    nc.compile()
    return nc


def _get_program(flags):
    if flags not in _PROGRAM_CACHE:
        _PROGRAM_CACHE[flags] = _build_program(flags)
    return _PROGRAM_CACHE[flags]


def kernel(
    x,
    ln1_w,
    ln1_b,
    Wqkv,
    bqkv,
    Wproj,
    bproj,
    Er,
    ln2_w,
    ln2_b,
    Wfc,
    bfc,
    Wfc2,
    bfc2,
):
    import ml_dtypes

    from concourse.bass_utils import run_bass_kernel_spmd

    bf = ml_dtypes.bfloat16
    x = np.asarray(x, np.float32)
    f = np.float32
    ntriv = lambda a, v: not np.all(np.asarray(a) == v)
    flags = (
        ntriv(ln1_w, 1) or ntriv(ln1_b, 0),
        ntriv(ln2_w, 1) or ntriv(ln2_b, 0),
        ntriv(bqkv[:D], 0),
        ntriv(bqkv[D : 2 * D], 0),
        ntriv(bqkv[2 * D :], 0),
        ntriv(bproj, 0),
        ntriv(bfc, 0),
        ntriv(bfc2, 0),
    )
    nc = _get_program(flags)

    ErT = np.asarray(Er, f).T * ERPRE           # [HS, L], 8x prescale
    ert2 = np.ascontiguousarray(
        np.concatenate([ErT, ErT], axis=0)
    ).astype(bf)
    c = np.ascontiguousarray
    Wqkv = np.asarray(Wqkv, f)
    Wfc_b = np.asarray(Wfc, f).astype(bf)
    Wfc2_b = np.asarray(Wfc2, f).astype(bf)
    in_maps = []
    for core in range(8):
        b, half = divmod(core, 2)
        hs0, hs1 = half * 512, (half + 1) * 512
        bq = np.asarray(bqkv[:D][hs0:hs1], f) * SCALE
        bk = np.asarray(bqkv[D : 2 * D][hs0:hs1], f)
        wqkv_half = np.concatenate(
            [
                Wqkv[:, 0:D][:, hs0:hs1],
                Wqkv[:, D : 2 * D][:, hs0:hs1],
                Wqkv[:, 2 * D :][:, hs0:hs1],
            ],
            axis=1,
        ).astype(bf)
        in_maps.append(
            {
                "x": c(x[b]).astype(bf),
                "x_my": c(x[b, hs0:hs1], f),
                "wqkv": c(wqkv_half),
                "wproj": c(np.asarray(Wproj, f)[hs0:hs1, :]).astype(bf),
                "ert2": ert2,
                "wfc": Wfc_b,
                "wfc2": Wfc2_b,
                "ln1a": c(np.asarray(ln1_w), f),
                "ln1b": c(np.asarray(ln1_b), f),
                "ln2a": c(np.asarray(ln2_w), f),
                "ln2b": c(np.asarray(ln2_b), f),
                "bq": c(bq.reshape(4, P).T, f),
                "bk": c(bk.reshape(4, P).T, f),
                "bv": c(np.asarray(bqkv[2 * D :][hs0:hs1]), f),
                "bproj": c(np.asarray(bproj), f),
                "bfc": c(np.asarray(bfc).reshape(FC, P).T, f),
                "bfc2": c(np.asarray(bfc2), f),
            }
        )

    trace = bool(int(os.environ.get("KERNEL_TRACE", "0")))
    res = run_bass_kernel_spmd(nc, in_maps, list(range(8)), trace=trace)
    global LAST_EXEC_NS, LAST_RESULT
    LAST_EXEC_NS = res.exec_time_ns
    LAST_RESULT = res
    out = np.empty((B, L, D), np.float32)
    for core in range(8):
        b, half = divmod(core, 2)
        out[b, half * 512 : (half + 1) * 512] = res.results[core]["out_my"]
    return out


LAST_EXEC_NS = None
LAST_RESULT = None
